# revision 1
# baseline (speedup 1.0000x reference)
import os
import sys

if "/opt/trn_rl_repo" not in sys.path:
    sys.path.insert(0, "/opt/trn_rl_repo")

import numpy as np

import concourse.bass as bass
import concourse.mybir as mybir
import concourse.tile as tile
from concourse import bacc
from concourse.bass_utils import run_bass_kernel_spmd

# Problem constants (hardcoded per harness contract).
B, Himg, Wimg, C, NH = 16, 56, 56, 128, 8
N = Himg * Wimg            # 3136
HD = C // NH               # 16
SCALE = HD ** -0.5         # 0.25
N4 = N // 4                # 784
TOK1 = (N // 49) // 4      # 16
TOK2 = (N // 14) // 2      # 112
TOK3 = (N // 7) // 4       # 112
TT = TOK1 + TOK2 + TOK3    # 240
F1, F2, F3 = 49, 14, 7
NCORES = 8
BPC = B // NCORES          # batches per core = 2
CH = 448                   # token chunk (448 = 8 rows of 56)
NCH = N // CH              # 7
RW = Wimg + 2              # padded row width 58
PADN = RW * (Himg + 2)     # 58*58 = 3364

FP32 = mybir.dt.float32
F32RT = mybir.dt.float32r
INT32 = mybir.dt.int32

# Head placement: all on-chip per-head blocks sit at 32-aligned partitions
# (HW requires 32-aligned partition starts; PE operands allow only 0/32/64).
# q/k tiles: A1 = heads 0,1,2 of branch1; A2 = heads 4,5,6 of branch2;
# B-tile "37": head 3 at rows 0:16, head 7 at rows 32:48.


STAGES = os.environ.get("STAGES", "ABCDEFGHIJ")


def _build_program():
    nc = bacc.Bacc(None, target_bir_lowering=False, debug=False)

    def din(name, shape, dt=FP32):
        return nc.dram_tensor(name, shape, dt, kind="ExternalInput")

    xin = din("xin", [BPC * N, C])
    idxin = din("idxin", [BPC * N, 1], INT32)
    wq012 = din("wq012", [C, C], F32RT)
    wq456 = din("wq456", [C, C], F32RT)
    wq37 = din("wq37", [C, 48], F32RT)
    wl = din("wl", [C, C], F32RT)
    wk1a = din("wk1a", [C, C])
    wk1b = din("wk1b", [C, 16])
    wv1a = din("wv1a", [C, C])
    wv1b = din("wv1b", [C, 16])
    wk2a = din("wk2a", [C, C])
    wk2b = din("wk2b", [C, 16])
    wv2a = din("wv2a", [C, C])
    wv2b = din("wv2b", [C, 16])
    wp0 = din("wp0", [C, C], F32RT)
    wp1 = din("wp1", [C, C], F32RT)
    wp2 = din("wp2", [48, C], F32RT)
    wpl = din("wpl", [C, C], F32RT)
    e8g0 = din("e8g0", [C, C], F32RT)
    e8g1 = din("e8g1", [C, C], F32RT)
    e8g2a = din("e8g2a", [C, 48], F32RT)
    e8g2b = din("e8g2b", [C, 48], F32RT)
    diag9 = din("diag9", [9 * C, C], F32RT)
    w1m = din("w1m", [112, 7 * TOK1])
    w2m = din("w2m", [112, 14 * TOK2])
    w3m = din("w3m", [112, 7 * TOK3])
    ident = din("ident", [C, C])
    bias3 = din("bias3", [C, 3])
    fb3 = din("fb3", [C, 3])
    outd = nc.dram_tensor("out", [BPC * N, C], FP32, kind="ExternalOutput")

    with tile.TileContext(nc) as tc:
        with (
            tc.tile_pool(name="const", bufs=1) as cpool,
            tc.tile_pool(name="big", bufs=1) as bigpool,
            tc.tile_pool(name="xtp", bufs=2) as xtpool,
            tc.tile_pool(name="xload", bufs=3) as xlpool,
            tc.tile_pool(name="gath", bufs=1) as gpool,
            tc.tile_pool(name="small", bufs=2) as smpool,
            tc.tile_pool(name="expp", bufs=4) as epool,
            tc.tile_pool(name="stream", bufs=2) as stpool,
            tc.tile_pool(name="attb", bufs=1) as abpool,
            tc.tile_pool(name="ps_mm", bufs=2, space="PSUM") as ps_mm,
            tc.tile_pool(name="ps_pl", bufs=3, space="PSUM") as ps_pl,
            tc.tile_pool(name="ps_po", bufs=2, space="PSUM") as ps_po,
            tc.tile_pool(name="ps_tp", bufs=1, space="PSUM") as ps_tp,
        ):
            # ---- constants to SBUF ----
            def cload(dram):
                shape = list(dram.shape)
                t = cpool.tile(shape, dram.dtype, tag=f"c_{dram.name}")
                nc.sync.dma_start(t[:, :], dram[:, :])
                return t

            wq012_s, wq456_s, wq37_s = map(cload, (wq012, wq456, wq37))
            wl_s = cload(wl)
            wk1a_s, wk1b_s, wv1a_s, wv1b_s = map(cload, (wk1a, wk1b, wv1a, wv1b))
            wk2a_s, wk2b_s, wv2a_s, wv2b_s = map(cload, (wk2a, wk2b, wv2a, wv2b))
            wp0_s, wp1_s, wp2_s, wpl_s = map(cload, (wp0, wp1, wp2, wpl))
            e8g0_s, e8g1_s, e8g2a_s, e8g2b_s = map(cload, (e8g0, e8g1, e8g2a, e8g2b))
            id_s = cload(ident)
            b3_s = cload(bias3)
            fb3_s = cload(fb3)
            w1m_s, w2m_s, w3m_s = map(cload, (w1m, w2m, w3m))
            diag_s = []
            for t in range(9):
                dt_ = cpool.tile([C, C], F32RT, tag=f"c_diag{t}")
                nc.sync.dma_start(dt_[:, :], diag9[t * C:(t + 1) * C, :])
                diag_s.append(dt_)

            F32R = mybir.dt.float32r

            def r(ap):
                return ap.bitcast(F32R)

            # persistent padded buffer for depthwise conv input (zero border)
            pad_t = bigpool.tile([C, PADN], F32RT, tag="pad")
            nc.vector.memset(pad_t[:, :].bitcast(FP32), 0.0)
            pad3 = pad_t[:, :].rearrange("p (r c) -> p r c", c=RW)

            # persistent attention staging buffers: pad rows only need
            # initializing once; real rows are rewritten every chunk.
            outp_bufs = []
            dn_bufs = []
            for par in range(2):
                op0 = abpool.tile([C, CH], FP32, tag=f"outp0_{par}", name="op0")
                op1 = abpool.tile([C, CH], FP32, tag=f"outp1_{par}", name="op1")
                op2 = abpool.tile([48, CH], FP32, tag=f"outp2_{par}", name="op2")
                for t_ in (op0, op1, op2):
                    nc.vector.memset(t_[:, :], 0.0)
                outp_bufs.append([op0, op1, op2])
                d1 = abpool.tile([C, CH], FP32, tag=f"dn1_{par}", name="d1")
                d2 = abpool.tile([C, CH], FP32, tag=f"dn2_{par}", name="d2")
                nc.vector.memset(d1[:, :], 1.0)
                nc.vector.memset(d2[:, :], 1.0)
                dn_bufs.append((d1, d2))


            for b in range(BPC):
                xb = b * N

                # ---- stage A: x^T  (28 transposed tiles of 112 rows) ----
                xT = xtpool.tile([C, N], F32RT, tag="xT")
                if "A" in STAGES:
                    xsrc = xin[:, :].rearrange("(t p) c -> p t c", p=112)
                    for g in range(7):
                        xt = xlpool.tile([112, 4 * C], FP32, tag="xt")
                        nc.sync.dma_start(
                            xt[:, :].rearrange("p (t c) -> p t c", c=C),
                            xsrc[:, 28 * b + 4 * g:28 * b + 4 * (g + 1), :])
                        tp = ps_tp.tile([C, 448], FP32, tag="tp")
                        for j in range(4):
                            nc.tensor.transpose(tp[:, 112 * j:112 * (j + 1)],
                                                xt[:, C * j:C * (j + 1)], id_s[:112, :112])
                        nc.vector.tensor_copy(xT[:, 448 * g:448 * (g + 1)], tp[:, :])

                
                # ---- stage B: padded q heads + lepe-linear into conv pad buf ----
                q012 = bigpool.tile([C, N], F32RT, tag="q012")
                q456 = bigpool.tile([C, N], F32RT, tag="q456")
                q37 = bigpool.tile([48, N], F32RT, tag="q37")
                if "B" in STAGES:
                    for c in range(NCH):
                        sl = slice(CH * c, CH * (c + 1))
                        for wt, dst in ((wq012_s, q012), (wq456_s, q456),
                                        (wq37_s, q37)):
                            m = wt.shape[1]
                            pq = ps_mm.tile([C, CH], FP32, tag="mm")
                            nc.tensor.matmul(out=pq[:m, :], lhsT=wt[:, :], rhs=xT[:, sl],
                                             start=True, stop=True)
                            nc.scalar.copy(dst[:m, sl], pq[:m, :])
                        pl = ps_mm.tile([C, CH], FP32, tag="mm")
                        nc.tensor.matmul(out=pl[:, :], lhsT=wl_s[:, :], rhs=xT[:, sl],
                                         start=True, stop=True)
                        pl3 = pl[:, :].rearrange("p (r c) -> p r c", c=Wimg)
                        nc.vector.tensor_scalar_add(
                            pad3[:, 1 + 8 * c:9 + 8 * c, 1:57], pl3, b3_s[:, 0:1])

                
                # ---- stage C: depthwise 3x3 conv via 9 diagonal matmuls ----
                lepeT = bigpool.tile([C, N], F32RT, tag="lepeT")
                if "C" in STAGES:
                    for c in range(NCH):
                        pc = ps_mm.tile([C, CH], FP32, tag="mm")
                        for t in range(9):
                            dy, dx = t // 3, t % 3
                            nc.tensor.matmul(
                                out=pc[:, :], lhsT=diag_s[t][:, :],
                                rhs=pad3[:, dy + 8 * c: dy + 8 * c + 8, dx: dx + Wimg],
                                start=(t == 0), stop=(t == 8))
                        nc.vector.tensor_scalar_add(
                            lepeT[:, CH * c:CH * (c + 1)], pc[:, :], b3_s[:, 1:2])

                
                # ---- stage D: index loads + token gathers ----
                if "D" in STAGES:
                    ptiles = []
                    for k in range(28):
                        it = xlpool.tile([112, 1], INT32, tag="it")
                        nc.sync.dma_start(it[:, :], idxin[xb + 112 * k: xb + 112 * (k + 1), :])
                        pt = gpool.tile([112, C], FP32, tag=f"p{k}")
                        nc.gpsimd.indirect_dma_start(
                            out=pt[:, :], out_offset=None, in_=xin[:, :],
                            in_offset=bass.IndirectOffsetOnAxis(ap=it[:, :1], axis=0))
                        ptiles.append(pt)
                    s2a = gpool.tile([128, C], FP32, tag="s2a")
                    s2b = gpool.tile([112, C], FP32, tag="s2b")
                    ita = xlpool.tile([128, 1], INT32, tag="ita")
                    nc.sync.dma_start(ita[:, :], idxin[xb + N - TT: xb + N - TT + 128, :])
                    nc.gpsimd.indirect_dma_start(
                        out=s2a[:, :], out_offset=None, in_=xin[:, :],
                        in_offset=bass.IndirectOffsetOnAxis(ap=ita[:, :1], axis=0))
                    itb = xlpool.tile([112, 1], INT32, tag="itb")
                    nc.sync.dma_start(itb[:, :], idxin[xb + N - 112: xb + N, :])
                    nc.gpsimd.indirect_dma_start(
                        out=s2b[:, :], out_offset=None, in_=xin[:, :],
                        in_offset=bass.IndirectOffsetOnAxis(ap=itb[:, :1], axis=0))

                
                # ---- stage E: seq1^T via block-weight matmuls ----
                if "E" in STAGES:
                    seq1T = smpool.tile([C, TT], FP32, tag="seq1T")
                    ps1 = ps_mm.tile([C, TT], FP32, tag="mm")
                    for k in range(7):
                        nc.tensor.matmul(out=ps1[:, 0:TOK1], lhsT=ptiles[k][:, :],
                                         rhs=w1m_s[:, TOK1 * k:TOK1 * (k + 1)],
                                         start=(k == 0), stop=(k == 6))
                    for k in range(14):
                        nc.tensor.matmul(out=ps1[:, TOK1:TOK1 + TOK2], lhsT=ptiles[7 + k][:, :],
                                         rhs=w2m_s[:, TOK2 * k:TOK2 * (k + 1)],
                                         start=(k == 0), stop=(k == 13))
                    for k in range(7):
                        nc.tensor.matmul(out=ps1[:, TOK1 + TOK2:TT], lhsT=ptiles[21 + k][:, :],
                                         rhs=w3m_s[:, TOK3 * k:TOK3 * (k + 1)],
                                         start=(k == 0), stop=(k == 6))
                    nc.vector.tensor_scalar_add(seq1T[:, 0:TOK1], ps1[:, 0:TOK1], fb3_s[:, 0:1])
                    nc.vector.tensor_scalar_add(seq1T[:, TOK1:TOK1 + TOK2],
                                                ps1[:, TOK1:TOK1 + TOK2], fb3_s[:, 1:2])
                    nc.vector.tensor_scalar_add(seq1T[:, TOK1 + TOK2:TT],
                                                ps1[:, TOK1 + TOK2:TT], fb3_s[:, 2:3])

                
                # ---- stage F: seq2^T (transpose the top-240 gather) ----
                if "F" in STAGES:
                    seq2T = smpool.tile([C, TT], FP32, tag="seq2T")
                    pss = ps_mm.tile([C, TT], FP32, tag="mm")
                    nc.tensor.transpose(pss[:, 0:128], s2a[:, :], id_s[:, :])
                    nc.tensor.transpose(pss[:, 128:TT], s2b[:, :], id_s[:112, :112])
                    nc.vector.tensor_copy(seq2T[:, :], pss[:, :])

                
                # ---- stage G: padded kv projections ----
                # kpa[br]: heads (0,1,2)|(4,5,6) at rows 0/32/64; k37/v37: head3@0, head7@32
                if "G" in STAGES:
                    kp1a = smpool.tile([C, TT], F32RT, tag="kp1a")
                    kp2a = smpool.tile([C, TT], F32RT, tag="kp2a")
                    vp1a = smpool.tile([C, TT], FP32, tag="vp1a")
                    vp2a = smpool.tile([C, TT], FP32, tag="vp2a")
                    k37 = smpool.tile([48, TT], F32RT, tag="k37")
                    v37 = smpool.tile([48, TT], FP32, tag="v37")
                    for (wt, seqT, dst, dp, m) in (
                            (wk1a_s, seq1T, kp1a, 0, C), (wv1a_s, seq1T, vp1a, 0, C),
                            (wk1b_s, seq1T, k37, 0, 16), (wv1b_s, seq1T, v37, 0, 16),
                            (wk2a_s, seq2T, kp2a, 0, C), (wv2a_s, seq2T, vp2a, 0, C),
                            (wk2b_s, seq2T, k37, 32, 16), (wv2b_s, seq2T, v37, 32, 16)):
                        pk = ps_mm.tile([C, TT], FP32, tag="mm")
                        nc.tensor.matmul(out=pk[:m, :], lhsT=wt[:, :], rhs=seqT[:, :],
                                         start=True, stop=True)
                        nc.vector.tensor_copy(dst[dp:dp + m, :], pk[:m, :])

                
                # ---- stage H: v^T -> ones-augmented v tiles ----
                # vaug[br][mc] [mlen, 136]: head hh at cols 34*hh: 0:16 v, 16:32 zero,
                # col 32 ones (-> psum row 32 = softmax denom), col 33 unused.
                if "H" in STAGES:
                    vaug = [[None, None], [None, None]]
                    for br in range(2):
                        vpa = vp1a if br == 0 else vp2a
                        for mc, (ms, ml) in enumerate(((0, 128), (128, 112))):
                            va = smpool.tile([128, 136], F32RT, tag=f"va{br}{mc}")
                            nc.vector.memset(va[:ml, :].bitcast(FP32), 0.0)
                            nc.vector.memset(va[:ml, 32::34].bitcast(FP32), 1.0)
                            vaug[br][mc] = va
                            for hh in range(4):
                                if hh < 3:
                                    src, sb = vpa, 32 * hh
                                else:
                                    src, sb = v37, 32 * br
                                pv = ps_tp.tile([C, 16], FP32, tag="tp")
                                nc.tensor.transpose(pv[:ml, :], src[sb:sb + 16, ms:ms + ml],
                                                    id_s[sb:sb + 16, sb:sb + 16])
                                nc.vector.tensor_copy(va[:ml, 34 * hh:34 * hh + 16], pv[:ml, :])

                
                # per-head operand locators: (k tile, q tile, base)
                def kq(h):
                    if h in (0, 1, 2):
                        return kp1a, q012, 32 * h
                    if h in (4, 5, 6):
                        return kp2a, q456, 32 * (h - 4)
                    return k37, q37, (0 if h == 3 else 32)

                # outp groups: g0=(0,1,2), g1=(4,5,6), g2=(3@0, 7@32)
                def og(h):
                    if h in (0, 1, 2):
                        return 0, 32 * h
                    if h in (4, 5, 6):
                        return 1, 32 * (h - 4)
                    return 2, (0 if h == 3 else 32)

                # ---- stage I+J: attention, software-pipelined ----
                def emit_logits(h, c):
                    sl = slice(CH * c, CH * (c + 1))
                    kt, qt, kb = kq(h)
                    pl0 = ps_pl.tile([C, CH], FP32, tag="pl", name="pl0")
                    nc.tensor.matmul(out=pl0[:, :], lhsT=r(kt[kb:kb + 16, 0:128]),
                                     rhs=r(qt[kb:kb + 16, sl]), start=True, stop=True)
                    pl1 = ps_pl.tile([112, CH], FP32, tag="pl", name="pl1")
                    nc.tensor.matmul(out=pl1[:, :], lhsT=r(kt[kb:kb + 16, 128:TT]),
                                     rhs=r(qt[kb:kb + 16, sl]), start=True, stop=True)
                    return pl0, pl1

                def emit_stage_I(c):
                    sl = slice(CH * c, CH * (c + 1))
                    outp = outp_bufs[c % 2]
                    dn1, dn2 = dn_bufs[c % 2]
                    pls = emit_logits(0, c)
                    for h in range(NH):
                        br = 0 if h < 4 else 1
                        hh = h % 4
                        pl0, pl1 = pls
                        if h + 1 < NH:
                            pls = emit_logits(h + 1, c)
                        e0 = epool.tile([C, CH], F32RT, tag="e0", name="e0")
                        nc.scalar.activation(e0[:, :], pl0[:, :],
                                             mybir.ActivationFunctionType.Exp)
                        e1 = epool.tile([112, CH], F32RT, tag="e1", name="e1")
                        nc.scalar.activation(e1[:, :], pl1[:, :],
                                             mybir.ActivationFunctionType.Exp)
                        po = ps_po.tile([33, CH], FP32, tag="po", name="po")
                        nc.tensor.matmul(out=po[:, :],
                                         lhsT=r(vaug[br][0][:, 34 * hh:34 * hh + 33]),
                                         rhs=r(e0[:, :]), start=True, stop=False)
                        nc.tensor.matmul(out=po[:, :],
                                         lhsT=r(vaug[br][1][:112, 34 * hh:34 * hh + 33]),
                                         rhs=r(e1[:, :]), start=False, stop=True)
                        g, gp = og(h)
                        if h % 2 == 0:
                            nc.vector.tensor_copy(outp[g][gp:gp + 16, :], po[0:16, :])
                        else:
                            nc.scalar.copy(outp[g][gp:gp + 16, :], po[0:16, :])
                        dn = dn1 if br == 0 else dn2
                        nc.vector.tensor_copy(dn[32 * hh:32 * hh + 1, :], po[32:33, :])
                    return outp, dn1, dn2

                def emit_stage_J(c, outp, dn1, dn2):
                    sl = slice(CH * c, CH * (c + 1))
                    rc1 = stpool.tile([C, CH], F32RT, tag="rc1", name="rc1")
                    rc2 = stpool.tile([C, CH], F32RT, tag="rc2", name="rc2")
                    with nc.allow_low_precision(reason="f32r softmax recip"):
                        nc.vector.reciprocal(rc1[:, :], dn1[:, :])
                        nc.vector.reciprocal(rc2[:, :], dn2[:, :])
                    prp = []
                    for g, lhs_list in enumerate((
                            ((e8g0_s, rc1),), ((e8g1_s, rc2),),
                            ((e8g2a_s, rc1), (e8g2b_s, rc2)))):
                        m = 48 if g == 2 else C
                        pg = ps_mm.tile([C, CH], FP32, tag="mm", name="pg")
                        for i_, (ew, rcx) in enumerate(lhs_list):
                            nc.tensor.matmul(out=pg[:m, :], lhsT=r(ew[:, :]), rhs=r(rcx[:, :]),
                                             start=(i_ == 0), stop=(i_ == len(lhs_list) - 1))
                        rpg = stpool.tile([C, CH], F32RT, tag=f"rp{g}")
                        nc.vector.tensor_tensor(out=rpg[:m, :], in0=outp[g][:m, :],
                                                in1=pg[:m, :], op=mybir.AluOpType.mult)
                        prp.append(rpg)
                    pp = ps_mm.tile([C, CH], FP32, tag="mm", name="pp")
                    nc.tensor.matmul(out=pp[:, :], lhsT=r(wp0_s[:, :]), rhs=r(prp[0][:, :]),
                                     start=True, stop=False)
                    nc.tensor.matmul(out=pp[:, :], lhsT=r(wp1_s[:, :]), rhs=r(prp[1][:, :]),
                                     start=False, stop=False)
                    nc.tensor.matmul(out=pp[:, :], lhsT=r(wp2_s[:, :]), rhs=r(prp[2][:48, :]),
                                     start=False, stop=False)
                    nc.tensor.matmul(out=pp[:, :], lhsT=r(wpl_s[:, :]), rhs=r(lepeT[:, sl]),
                                     start=False, stop=True)
                    sp = stpool.tile([C, CH], FP32, tag="sp", name="sp")
                    nc.vector.tensor_scalar_add(sp[:, :], pp[:, :], b3_s[:, 2:3])
                    so = xlpool.tile([112, 4 * 128], FP32, tag="so", name="so")
                    pt2 = ps_po.tile([112, 4 * 128], FP32, tag="po", name="pt2")
                    for j in range(4):
                        nc.tensor.transpose(pt2[:, 128 * j:128 * (j + 1)],
                                            sp[:, 112 * j:112 * (j + 1)], id_s[:, :])
                    nc.vector.tensor_copy(so[:, :], pt2[:, :])
                    nc.sync.dma_start(
                        outd[:, :].rearrange("(t p) c -> p t c", p=112)[
                            :, (xb + CH * c) // 112:(xb + CH * c) // 112 + 4, :],
                        so[:, :].rearrange("p (t c) -> p t c", c=128))

                if "I" in STAGES:
                    prev = emit_stage_I(0)
                    for c in range(1, NCH):
                        cur = emit_stage_I(c)
                        emit_stage_J(c - 1, *prev)
                        prev = cur
                    emit_stage_J(NCH - 1, *prev)

    nc.compile()
    return nc


def _host_consts(W_q, W_kv1, W_kv2, lepe_lin_w, lepe_lin_b, lepe_conv_w, lepe_conv_b,
                 proj_w, proj_b, f1_w, f1_b, f2_w, f2_b, f3_w, f3_b):
    cc = np.ascontiguousarray
    f32 = np.float32
    consts = {}
    Wq = np.asarray(W_q, f32) * SCALE          # (C_out, C_in)
    Wk1 = np.asarray(W_kv1, f32)
    Wk2 = np.asarray(W_kv2, f32)
    Pw = np.asarray(proj_w, f32)

    def padheads(Wrows, heads):
        # lhsT [C_in, padded cols (16 used per 32-stride head)] producing padded rows
        out = np.zeros((C, C if len(heads) == 3 else 32 * len(heads)), f32)
        for lh, h in enumerate(heads):
            out[:, 32 * lh:32 * lh + 16] = Wrows[16 * h:16 * h + 16, :].T
        return cc(out)

    consts["wq012"] = padheads(Wq, (0, 1, 2))
    consts["wq456"] = padheads(Wq, (4, 5, 6))
    wq37 = np.zeros((C, 48), f32)
    wq37[:, 0:16] = Wq[48:64, :].T
    wq37[:, 32:48] = Wq[112:128, :].T
    consts["wq37"] = cc(wq37)
    consts["wl"] = cc(np.asarray(lepe_lin_w, f32).T.copy())
    # kv linear output channel z*64 + 16*hh + d ; branch heads hh=0..3
    for br, Wk in ((1, Wk1), (2, Wk2)):
        consts[f"wk{br}a"] = padheads(Wk[0:64, :], (0, 1, 2))
        consts[f"wk{br}b"] = cc(Wk[48:64, :].T.copy())
        consts[f"wv{br}a"] = padheads(Wk[64:128, :], (0, 1, 2))
        consts[f"wv{br}b"] = cc(Wk[112:128, :].T.copy())
    # padded projection weights: lhsT rows = padded (group-local) channel rows
    def projpad(heads, rows):
        out = np.zeros((rows, C), f32)
        for lh, h in enumerate(heads):
            out[32 * lh:32 * lh + 16, :] = Pw[:, 16 * h:16 * h + 16].T
        return cc(out)
    consts["wp0"] = projpad((0, 1, 2), C)
    consts["wp1"] = projpad((4, 5, 6), C)
    consts["wp2"] = projpad((3, 7), 48)
    consts["wpl"] = cc(Pw.T.copy())
    # recip-broadcast selectors: lhsT [dn rows, outp-group rows]
    def esel(pairs, rows):
        out = np.zeros((C, rows), f32)
        for dnrow, grow in pairs:
            out[dnrow, grow:grow + 16] = 1.0
        return cc(out)
    consts["e8g0"] = esel([(0, 0), (32, 32), (64, 64)], C)
    consts["e8g1"] = esel([(0, 0), (32, 32), (64, 64)], C)
    consts["e8g2a"] = esel([(96, 0)], 48)
    consts["e8g2b"] = esel([(96, 32)], 48)
    d9 = np.zeros((9 * C, C), f32)
    cw = np.asarray(lepe_conv_w, f32)  # (C,1,3,3)
    for t in range(9):
        d9[t * C + np.arange(C), np.arange(C)] = cw[:, 0, t // 3, t % 3]
    consts["diag9"] = d9

    def blockw(L, tok, f, fw):
        w = np.zeros((L, tok), f32)
        fw = np.asarray(fw, f32).reshape(-1)
        for g in range(tok):
            w[g * f:(g + 1) * f, g] = fw
        nch = L // 112
        return cc(w.reshape(nch, 112, tok).transpose(1, 0, 2).reshape(112, nch * tok))

    consts["w1m"] = blockw(N4, TOK1, F1, f1_w)
    consts["w2m"] = blockw(2 * N4, TOK2, F2, f2_w)
    consts["w3m"] = blockw(N4, TOK3, F3, f3_w)
    consts["ident"] = np.eye(C, dtype=f32)
    b3 = np.zeros((C, 3), f32)
    b3[:, 0] = np.asarray(lepe_lin_b, f32).reshape(-1)
    b3[:, 1] = np.asarray(lepe_conv_b, f32).reshape(-1)
    b3[:, 2] = np.asarray(proj_b, f32).reshape(-1)
    consts["bias3"] = b3
    fb = np.zeros((C, 3), f32)
    fb[:, 0] = f32(np.asarray(f1_b).reshape(-1)[0])
    fb[:, 1] = f32(np.asarray(f2_b).reshape(-1)[0])
    fb[:, 2] = f32(np.asarray(f3_b).reshape(-1)[0])
    consts["fb3"] = fb
    return consts


_RUN_KW = {}


def kernel(x, mask, H, W, W_q, W_kv1, W_kv2, f1_w, f1_b, f2_w, f2_b, f3_w, f3_b,
           lepe_lin_w, lepe_lin_b, lepe_conv_w, lepe_conv_b, proj_w, proj_b):
    x = np.ascontiguousarray(np.asarray(x, dtype=np.float32))
    mask = np.asarray(mask, dtype=np.float32)
    idx = np.argsort(mask.reshape(B, N), axis=1, kind="stable").astype(np.int32)

    consts = _host_consts(W_q, W_kv1, W_kv2, lepe_lin_w, lepe_lin_b, lepe_conv_w,
                          lepe_conv_b, proj_w, proj_b, f1_w, f1_b, f2_w, f2_b,
                          f3_w, f3_b)

    nc = _build_program()

    in_maps = []
    for core in range(NCORES):
        bs = core * BPC
        xloc = np.ascontiguousarray(x[bs:bs + BPC].reshape(BPC * N, C))
        iloc = (idx[bs:bs + BPC] + (np.arange(BPC)[:, None] * N).astype(np.int32))
        iloc = np.ascontiguousarray(iloc.reshape(BPC * N, 1))
        m = {"xin": xloc, "idxin": iloc}
        m.update(consts)
        in_maps.append(m)

    res = run_bass_kernel_spmd(nc, in_maps, core_ids=list(range(NCORES)), **_RUN_KW)
    out = np.empty((B, N, C), np.float32)
    for core in range(NCORES):
        bs = core * BPC
        out[bs:bs + BPC] = res.results[core]["out"].reshape(BPC, N, C)
    kernel.last_result = res
    return out



# revision 63
# speedup vs baseline: 1.5392x; 1.5392x over previous
import os
import sys

if "/opt/trn_rl_repo" not in sys.path:
    sys.path.insert(0, "/opt/trn_rl_repo")

import ml_dtypes
import numpy as np

import concourse.bass as bass
import concourse.mybir as mybir
import concourse.tile as tile
from concourse import bacc
from concourse.bass_utils import run_bass_kernel_spmd

# Problem constants (hardcoded per harness contract).
B, Himg, Wimg, C, NH = 16, 56, 56, 128, 8
N = Himg * Wimg            # 3136
HD = C // NH               # 16
SCALE = HD ** -0.5         # 0.25
N4 = N // 4                # 784
TOK1 = (N // 49) // 4      # 16
TOK2 = (N // 14) // 2      # 112
TOK3 = (N // 7) // 4       # 112
TT = TOK1 + TOK2 + TOK3    # 240
F1, F2, F3 = 49, 14, 7
NCORES = 8
BPC = B // NCORES          # batches per core = 2
CH = 448                   # token chunk (448 = 8 rows of 56)
NCH = N // CH              # 7
RW = Wimg + 2              # padded row width 58
PADN = RW * (Himg + 2)     # 58*58 = 3364

FP32 = mybir.dt.float32
F32RT = mybir.dt.float32r
BF16 = mybir.dt.bfloat16
INT32 = mybir.dt.int32

# Head placement: all on-chip per-head blocks sit at 32-aligned partitions
# (HW requires 32-aligned partition starts; PE operands allow only 0/32/64).
# q/k tiles: A1 = heads 0,1,2 of branch1; A2 = heads 4,5,6 of branch2;
# B-tile "37": head 3 at rows 0:16, head 7 at rows 32:48.
# Attention epilogue: per-branch PSUM tile [128,448]; head hh of the branch
# occupies rows 32*hh..32*hh+15 (numerators) and row 32*hh+16 (softmax
# denominator via a ones column in the v tile); rows 17..31 of each 32-block
# are never written and hold 1.0 from a one-time memset.


STAGES = os.environ.get("STAGES", "ABCDEFGHIJ")


def _cat_layout(entries):
    off, table = 0, {}
    for name, w, rows in entries:
        table[name] = (off, w, rows)
        off += w
    return table, off


CATG_OFF, CATG_COLS = _cat_layout(
    [("wk1a", C, C), ("wk1b", 16, C), ("wv1a", C, C), ("wv1b", 16, C),
     ("wk2a", C, C), ("wk2b", 16, C), ("wv2a", C, C), ("wv2b", 16, C),
     ("wqTA", 3 * C, C), ("wqTB", 3 * C, C), ("wqTC", 2 * C, 48)])
CATJ_OFF, CATJ_COLS = _cat_layout(
    [("wpA", C, C), ("wpB", C, C), ("wpl", C, C), ("e8n", C, C)]
    + [(f"diag{t}", C, C) for t in range(9)])
CATH_OFF, CATH_COLS = _cat_layout([("ident16", C, C), ("wl16", C, C)])
S1W = 28 * TT


def _build_program():
    nc = bacc.Bacc(None, target_bir_lowering=False, debug=False)

    def din(name, shape, dt=FP32):
        return nc.dram_tensor(name, shape, dt, kind="ExternalInput")

    xin = din("xin", [BPC * N, C], BF16)
    idxin = din("idxin", [BPC * N, 1], INT32)
    catg = din("catg", [C, CATG_COLS], BF16)
    catj = din("catj", [C, CATJ_COLS], F32RT)
    cath = din("cath", [C, CATH_COLS], BF16)
    biasid = din("biasid", [C, 6 + C + 9], FP32)
    s1in = din("s1in", [112, BPC * S1W], BF16)
    outd = nc.dram_tensor("out", [BPC * N, C], BF16, kind="ExternalOutput")

    with tile.TileContext(nc) as tc:
        with (
            tc.tile_pool(name="const", bufs=1) as cpool,
            tc.tile_pool(name="big", bufs=1) as bigpool,
            tc.tile_pool(name="xtp", bufs=2) as xtpool,
            tc.tile_pool(name="xload", bufs=2) as xlpool,
            tc.tile_pool(name="xbig", bufs=2) as xbpool,
            tc.tile_pool(name="gath", bufs=1) as gpool,
            tc.tile_pool(name="small", bufs=2) as smpool,
            tc.tile_pool(name="expp", bufs=8) as epool,
            tc.tile_pool(name="opp", bufs=1) as oppool,
            tc.tile_pool(name="stream", bufs=2) as stpool,
            tc.tile_pool(name="ps_mm", bufs=2, space="PSUM") as ps_mm,
            tc.tile_pool(name="ps_pl", bufs=2, space="PSUM") as ps_pl,
            tc.tile_pool(name="ps_s1", bufs=1, space="PSUM") as ps_s1,
            tc.tile_pool(name="ps_po", bufs=2, space="PSUM") as ps_po,
            tc.tile_pool(name="ps_tp", bufs=1, space="PSUM") as ps_tp,
        ):
            # ---- constants: early (bf16 ident/wl + kv/q weights) now,
            # late (projection/conv weights) after the x/s1 loads ----
            ch_t = cpool.tile([C, CATH_COLS], BF16, tag="cath")
            cg_t = cpool.tile([C, CATG_COLS], BF16, tag="catg")
            cj_t = cpool.tile([C, CATJ_COLS], F32RT, tag="catj")
            bi_t = cpool.tile([C, 6 + C + 9], FP32, tag="c_b6")

            def emit_early_consts():
                nc.sync.dma_start(ch_t[:, :], cath[:, :])
                nc.sync.dma_start(cg_t[:, :], catg[:, :])

            def emit_late_consts():
                nc.sync.dma_start(bi_t[:, :], biasid[:, :])
                nc.sync.dma_start(cj_t[:, :], catj[:, :])

            def slC(tile_, table, name):
                o, w, rows = table[name]
                return tile_[:rows, o:o + w]

            id16_s = slC(ch_t, CATH_OFF, "ident16")
            wl_s = slC(ch_t, CATH_OFF, "wl16")
            wqTA_s, wqTB_s, wqTC_s = (slC(cg_t, CATG_OFF, n)
                                      for n in ("wqTA", "wqTB", "wqTC"))
            wk1a_s, wk1b_s = slC(cg_t, CATG_OFF, "wk1a"), slC(cg_t, CATG_OFF, "wk1b")
            wv1a_s, wv1b_s = slC(cg_t, CATG_OFF, "wv1a"), slC(cg_t, CATG_OFF, "wv1b")
            wk2a_s, wk2b_s = slC(cg_t, CATG_OFF, "wk2a"), slC(cg_t, CATG_OFF, "wk2b")
            wv2a_s, wv2b_s = slC(cg_t, CATG_OFF, "wv2a"), slC(cg_t, CATG_OFF, "wv2b")
            wpA_s, wpB_s, wpl_s = (slC(cj_t, CATJ_OFF, n)
                                   for n in ("wpA", "wpB", "wpl"))
            e8n_s = slC(cj_t, CATJ_OFF, "e8n")
            diag_s = [slC(cj_t, CATJ_OFF, f"diag{t}") for t in range(9)]
            b3_s = bi_t[:, 0:3]
            fb3_s = bi_t[:, 3:6]
            idf_s = bi_t[:, 6:6 + C]
            cw9_s = bi_t[:, 6 + C:6 + C + 9]

            F32R = mybir.dt.float32r

            def r(ap):
                return ap.bitcast(F32R)

            # persistent padded buffer for depthwise conv input (zero border)
            pad_t = bigpool.tile([C, PADN], F32RT, tag="pad")
            pad3 = pad_t[:, :].rearrange("p (r c) -> p r c", c=RW)

            # persistent per-branch attention accumulator tiles in SBUF;
            # rows 17..31 of each 32-block are never written and keep 1.0 so
            # the whole-tile reciprocal in stage J stays finite.
            op_att = [oppool.tile([C, CH], F32RT, tag=f"op{br}", name=f"opt{br}")
                      for br in range(2)]

            def emit_persistent_memsets():
                nc.gpsimd.memset(pad_t[:, :].bitcast(FP32), 0.0)
                for t_ in op_att:
                    nc.gpsimd.memset(t_[:, :].bitcast(FP32), 1.0)

            cp = nc.vector.tensor_copy

            def emit_pro_a(b, S):
                """Gather-dependent branch-2 chain + first x^T group."""
                xb = b * N
                S["xb"] = xb
                # ---- stage D: seq2 top-240 token gathers ----
                s2a = gpool.tile([128, C], BF16, tag="s2a")
                s2b = gpool.tile([112, C], BF16, tag="s2b")
                ita = xlpool.tile([128, 1], INT32, tag="ita")
                nc.sync.dma_start(ita[:, :], idxin[xb + N - TT: xb + N - TT + 128, :])
                nc.gpsimd.indirect_dma_start(
                    out=s2a[:, :], out_offset=None, in_=xin[:, :],
                    in_offset=bass.IndirectOffsetOnAxis(ap=ita[:, :1], axis=0))
                itb = xlpool.tile([112, 1], INT32, tag="itb")
                nc.sync.dma_start(itb[:, :], idxin[xb + N - 112: xb + N, :])
                nc.gpsimd.indirect_dma_start(
                    out=s2b[:, :], out_offset=None, in_=xin[:, :],
                    in_offset=bass.IndirectOffsetOnAxis(ap=itb[:, :1], axis=0))
                yield
                # ---- x / S1 loads ----
                xT = xtpool.tile([C, N], BF16, tag="xT")
                xsrc = xin[:, :].rearrange("(t p) c -> p t c", p=112)
                xt = xbpool.tile([112, 28 * C], BF16, tag="xt")
                s1_s = cpool.tile([112, S1W], BF16, tag="c_s1")
                for dq in range(4):
                    nc.sync.dma_start(
                        xt[:, 7 * C * dq:7 * C * (dq + 1)].rearrange(
                            "p (t c) -> p t c", c=C),
                        xsrc[:, 28 * b + 7 * dq:28 * b + 7 * (dq + 1), :])
                    lo = dq * 7 * TT
                    hi = min(S1W, (dq + 1) * 7 * TT)
                    nc.sync.dma_start(s1_s[:, lo:hi],
                                      s1in[:, b * S1W + lo:b * S1W + hi])
                    if dq == 0:
                        yield
                S.update(xT=xT, xt=xt, s1_s=s1_s, s2a=s2a, s2b=s2b)
                yield

                def emit_xtg(g):
                    tp = ps_tp.tile([C, 448], BF16, tag="tp")
                    for j in range(4):
                        t_ = 4 * g + j
                        nc.tensor.transpose(tp[:, 112 * j:112 * (j + 1)],
                                            xt[:, C * t_:C * (t_ + 1)],
                                            id16_s[:112, :112])
                    cp(xT[:, 448 * g:448 * (g + 1)], tp[:, :])
                S["emit_xtg"] = emit_xtg
                emit_xtg(0)
                # ---- stage F: seq2^T ----
                seq2T = smpool.tile([C, 256], BF16, tag="seq2T")
                nc.vector.memset(seq2T[:, 240:256], 0.0)
                pss = ps_tp.tile([C, TT], BF16, tag="tp")
                nc.tensor.transpose(pss[:, 0:128], s2a[:, :], id16_s[:, :])
                nc.tensor.transpose(pss[:, 128:TT], s2b[:, :], id16_s[:112, :112])
                cp(seq2T[:, 0:TT], pss[:, :])
                yield
                kp1a = smpool.tile([C, 256], BF16, tag="kp1a")
                kp2a = smpool.tile([C, 256], BF16, tag="kp2a")
                vp1a = smpool.tile([C, TT], FP32, tag="vp1a")
                vp2a = smpool.tile([C, TT], FP32, tag="vp2a")
                k37 = smpool.tile([48, 256], BF16, tag="k37")
                v37 = smpool.tile([48, TT], FP32, tag="v37")
                kqs = smpool.tile([C, NH * TT], BF16, tag="kqs")
                vaug = [[None, None], [None, None]]
                S.update(kp1a=kp1a, kp2a=kp2a, vp1a=vp1a, vp2a=vp2a, k37=k37,
                         v37=v37, kqs=kqs, vaug=vaug, seq1T=None, seq2T=seq2T)

                def emit_G(projs):
                    for (wt, seqT, dst, dp, m) in projs:
                        pk = ps_mm.tile([C, 256], FP32, tag="mm")
                        nc.tensor.matmul(out=pk[:m, :], lhsT=wt[:, :],
                                         rhs=seqT[:, 0:256], start=True, stop=True)
                        cp(dst[dp:dp + m, 0:TT], pk[:m, 0:TT])

                def emit_G2(heads):
                    # kq_h = (scale*Wq_h)^T k_h, so logits = kq_h^T xT
                    for h in heads:
                        hh = h % 4
                        if hh < 3:
                            kt = kp1a if h < 4 else kp2a
                            wt = wqTA_s if h < 4 else wqTB_s
                            kb, wc = 32 * hh, C * hh
                        else:
                            kt = k37
                            wt = wqTC_s
                            kb, wc = 32 * (h // 4), C * (h // 4)
                        pq = ps_mm.tile([C, 256], FP32, tag="mm", name="kq")
                        nc.tensor.matmul(out=pq[:, :],
                                         lhsT=wt[kb:kb + 16, wc:wc + C],
                                         rhs=kt[kb:kb + 16, 0:256],
                                         start=True, stop=True)
                        cp(kqs[:, TT * h:TT * (h + 1)], pq[:, 0:TT])

                def emit_H(br):
                    # vaug[br][mc] [mlen, 136]: head hh at cols 34*hh: 0:16 v,
                    # col 16 ones (-> denominator row 16 of the po matmul),
                    # cols 17:33 zero so a 33-wide lhsT keeps the PE tile legal.
                    vpa = vp1a if br == 0 else vp2a
                    for mc, (ms, ml) in enumerate(((0, 128), (128, 112))):
                        va = smpool.tile([128, 136], F32RT, tag=f"va{br}{mc}")
                        nc.gpsimd.memset(va[:ml, :].bitcast(FP32), 0.0)
                        nc.gpsimd.memset(va[:ml, 16::34].bitcast(FP32), 1.0)
                        vaug[br][mc] = va
                        for hh in range(4):
                            if hh < 3:
                                vsrc, sb = vpa, 32 * hh
                            else:
                                vsrc, sb = v37, 32 * br
                            pv = ps_po.tile([C, 16], FP32, tag="po", name="pv")
                            nc.tensor.transpose(pv[:ml, :],
                                                vsrc[sb:sb + 16, ms:ms + ml],
                                                idf_s[sb:sb + 16, sb:sb + 16])
                            cp(va[:ml, 34 * hh:34 * hh + 16], pv[:ml, :])
                S.update(emit_G=emit_G, emit_G2=emit_G2, emit_H=emit_H)
                # branch 2 (heads 4-7) only needs seq2T
                emit_G(((wk2a_s, seq2T, kp2a, 0, C), (wv2a_s, seq2T, vp2a, 0, C),
                        (wk2b_s, seq2T, k37, 32, 16), (wv2b_s, seq2T, v37, 32, 16)))
                yield
                emit_G2((4, 5))
                emit_G2((6, 7))
                yield
                emit_H(1)
                S["lepeT"] = bigpool.tile([C, N], F32RT, tag="lepeT", name="lepeT")

            def emit_pro_b(b, S):
                """S1-gated seq1 chain: remaining x^T groups, seq1, branch-1
                kv/kq/vaug. Drained as filler inside the chunk stream."""
                xt, s1_s = S["xt"], S["s1_s"]
                for g in range(1, 7):
                    S["emit_xtg"](g)
                    yield
                seq1T = smpool.tile([C, 256], BF16, tag="seq1T")
                S["seq1T"] = seq1T
                ps1 = ps_s1.tile([C, TT], FP32, tag="s1", name="ps1")
                for g in range(7):
                    for j in range(4):
                        t_ = 4 * g + j
                        nc.tensor.matmul(
                            out=ps1[:, :], lhsT=xt[:, C * t_:C * (t_ + 1)],
                            rhs=s1_s[:, TT * t_:TT * (t_ + 1)],
                            start=(t_ == 0), stop=(t_ == 27))
                    yield
                nc.vector.tensor_scalar_add(seq1T[:, 0:TOK1], ps1[:, 0:TOK1],
                                            fb3_s[:, 0:1])
                nc.vector.tensor_scalar_add(seq1T[:, TOK1:TOK1 + TOK2],
                                            ps1[:, TOK1:TOK1 + TOK2], fb3_s[:, 1:2])
                nc.vector.tensor_scalar_add(seq1T[:, TOK1 + TOK2:TT],
                                            ps1[:, TOK1 + TOK2:TT], fb3_s[:, 2:3])
                nc.vector.memset(seq1T[:, 240:256], 0.0)
                yield
                S["emit_G"](((wk1a_s, seq1T, S["kp1a"], 0, C),
                             (wv1a_s, seq1T, S["vp1a"], 0, C),
                             (wk1b_s, seq1T, S["k37"], 0, 16),
                             (wv1b_s, seq1T, S["v37"], 0, 16)))
                yield
                S["emit_G2"]((0, 1))
                yield
                S["emit_G2"]((2, 3))
                yield
                S["emit_H"](0)

            def chain2(g1, g2):
                yield from g1
                yield from g2

            def step(filler, n=1):
                if filler is None:
                    return
                for _ in range(n):
                    try:
                        next(filler)
                    except StopIteration:
                        return

            def emit_B(S, c):
                sl = slice(CH * c, CH * (c + 1))
                pl = ps_mm.tile([C, CH], FP32, tag="mm")
                nc.tensor.matmul(out=pl[:, :], lhsT=wl_s[:, :], rhs=S["xT"][:, sl],
                                 start=True, stop=True)
                pl3 = pl[:, :].rearrange("p (r c) -> p r c", c=Wimg)
                nc.vector.tensor_scalar_add(
                    pad3[:, 1 + 8 * c:9 + 8 * c, 1:57], pl3, b3_s[:, 0:1])

            def emit_C(S, c):
                pc = ps_mm.tile([C, CH], FP32, tag="mm")
                for t in range(9):
                    dy, dx = t // 3, t % 3
                    nc.tensor.matmul(
                        out=pc[:, :], lhsT=diag_s[t][:, :],
                        rhs=pad3[:, dy + 8 * c: dy + 8 * c + 8, dx: dx + Wimg],
                        start=(t == 0), stop=(t == 8))
                nc.vector.tensor_scalar_add(
                    S["lepeT"][:, CH * c:CH * (c + 1)], pc[:, :], b3_s[:, 1:2])

            def emit_logits(S, h, c):
                sl = slice(CH * c, CH * (c + 1))
                kqs = S["kqs"]
                pl0 = ps_pl.tile([C, CH], FP32, tag="pl", name="pl0")
                nc.tensor.matmul(out=pl0[:, :], lhsT=kqs[:, TT * h:TT * h + 128],
                                 rhs=S["xT"][:, sl], start=True, stop=True)
                pl1 = ps_pl.tile([112, CH], FP32, tag="pl", name="pl1")
                nc.tensor.matmul(out=pl1[:, :], lhsT=kqs[:, TT * h + 128:TT * (h + 1)],
                                 rhs=S["xT"][:, sl], start=True, stop=True)
                return pl0, pl1

            def emit_J_head(br):
                # Softmax division for a completed branch: reciprocal of
                # the assembled op tile -> row-broadcast matmul -> multiply.
                op = op_att[br]
                rc = stpool.tile([C, CH], F32RT, tag=f"rc{br}", name="rc")
                with nc.allow_low_precision(reason="f32r softmax recip"):
                    nc.vector.reciprocal(rc[:, :], op[:, :])
                pg = ps_mm.tile([C, CH], FP32, tag="mm", name="pg")
                nc.tensor.matmul(out=pg[:, :], lhsT=e8n_s[:, :], rhs=rc[:, :],
                                 start=True, stop=True)
                rpg = stpool.tile([C, CH], F32RT, tag=f"rp{br}")
                nc.vector.tensor_tensor(out=rpg[:, :], in0=op[:, :],
                                        in1=pg[:, :], op=mybir.AluOpType.mult)
                return rpg

            HORDER = (4, 5, 6, 7, 0, 1, 2, 3)

            def emit_chunk_I(S, c, pls, filler, boost=False):
                vaug = S["vaug"]
                prp = [None, None]
                for hi in range(NH):
                    h = HORDER[hi]
                    br = 0 if h < 4 else 1
                    hh = h % 4
                    pl0, pl1 = pls
                    if hi + 1 < NH:
                        pls = emit_logits(S, HORDER[hi + 1], c)
                    e0 = epool.tile([C, CH], F32RT, tag="e0", name="e0")
                    nc.scalar.activation(e0[:, :], pl0[:, :],
                                         mybir.ActivationFunctionType.Exp)
                    e1 = epool.tile([112, CH], F32RT, tag="e1", name="e1")
                    nc.scalar.activation(e1[:, :], pl1[:, :],
                                         mybir.ActivationFunctionType.Exp)
                    po = ps_po.tile([33, CH], FP32, tag="po", name="po")
                    nc.tensor.matmul(out=po[:, :],
                                     lhsT=vaug[br][0][:, 34 * hh:34 * hh + 33],
                                     rhs=e0[:, :], start=True, stop=False)
                    nc.tensor.matmul(out=po[:, :],
                                     lhsT=vaug[br][1][:112, 34 * hh:34 * hh + 33],
                                     rhs=e1[:, :], start=False, stop=True)
                    if h == 5:
                        nc.scalar.copy(
                            op_att[br][32 * hh:32 * hh + 17, :], po[0:17, :])
                    else:
                        nc.vector.tensor_copy(
                            op_att[br][32 * hh:32 * hh + 17, :], po[0:17, :])
                    if hh == 3:
                        prp[br] = emit_J_head(br)
                        if br == 0 and c + 1 < NCH:
                            pls = emit_logits(S, HORDER[0], c + 1)
                    if boost and hi < 3:
                        step(filler, 6)
                    elif h % 2 == 1:
                        step(filler)
                return prp, pls

            def emit_J_tail(S, c, prp):
                sl = slice(CH * c, CH * (c + 1))
                pp = ps_mm.tile([C, CH], FP32, tag="mm", name="pp")
                nc.tensor.matmul(out=pp[:, :], lhsT=wpA_s[:, :], rhs=r(prp[0][:, :]),
                                 start=True, stop=False)
                nc.tensor.matmul(out=pp[:, :], lhsT=wpB_s[:, :], rhs=r(prp[1][:, :]),
                                 start=False, stop=False)
                nc.tensor.matmul(out=pp[:, :], lhsT=wpl_s[:, :], rhs=S["lepeT"][:, sl],
                                 start=False, stop=True)
                sp = oppool.tile([C, CH], BF16, tag="sp", name="sp")
                nc.vector.tensor_scalar_add(sp[:, :], pp[:, :], b3_s[:, 2:3])
                so = xlpool.tile([112, 4 * 128], BF16, tag="so", name="so")
                pt2 = ps_tp.tile([112, 4 * 128], BF16, tag="tp", name="pt2")
                for j in range(4):
                    nc.tensor.transpose(pt2[:, 128 * j:128 * (j + 1)],
                                        sp[:, 112 * j:112 * (j + 1)], id16_s[:, :])
                nc.vector.tensor_copy(so[:, :], pt2[:, :])
                nc.sync.dma_start(
                    outd[:, :].rearrange("(t p) c -> p t c", p=112)[
                        :, (S["xb"] + CH * c) // 112:(S["xb"] + CH * c) // 112 + 4, :],
                    so[:, :].rearrange("p (t c) -> p t c", c=128))

            def emit_chunks(S, filler, boost=False):
                emit_B(S, 0)
                emit_B(S, 1)
                pls = emit_logits(S, HORDER[0], 0)
                for c in range(NCH):
                    if c == NCH - 1:
                        emit_C(S, c)
                    prp, pls = emit_chunk_I(S, c, pls, filler, boost and c == 0)
                    if c + 2 < NCH:
                        emit_B(S, c + 2)
                    if c < NCH - 1:
                        emit_C(S, c)
                    emit_J_tail(S, c, prp)
                    step(filler)

            S0, S1 = {}, {}
            gen0a = emit_pro_a(0, S0)
            step(gen0a, 1)
            emit_persistent_memsets()
            step(gen0a, 1)
            emit_early_consts()
            step(gen0a, 1)
            emit_late_consts()
            step(gen0a, 10 ** 6)
            step(emit_pro_b(0, S0), 10 ** 6)
            fill0 = chain2(emit_pro_a(1, S1), emit_pro_b(1, S1))
            emit_chunks(S0, fill0, boost=False)
            step(fill0, 10 ** 6)
            emit_chunks(S1, None)

    nc.compile()
    return nc


def _host_consts(W_q, W_kv1, W_kv2, lepe_lin_w, lepe_lin_b, lepe_conv_w, lepe_conv_b,
                 proj_w, proj_b, f1_w, f1_b, f2_w, f2_b, f3_w, f3_b):
    cc = np.ascontiguousarray
    f32 = np.float32
    bf16 = ml_dtypes.bfloat16
    consts = {}
    Wq = np.asarray(W_q, f32) * SCALE          # (C_out, C_in)
    Wk1 = np.asarray(W_kv1, f32)
    Wk2 = np.asarray(W_kv2, f32)
    Pw = np.asarray(proj_w, f32)

    def padheads(Wrows, heads):
        # lhsT [C_in, padded cols (16 used per 32-stride head)] producing padded rows
        out = np.zeros((C, C if len(heads) == 3 else 32 * len(heads)), f32)
        for lh, h in enumerate(heads):
            out[:, 32 * lh:32 * lh + 16] = Wrows[16 * h:16 * h + 16, :].T
        return cc(out)

    # wqT blocks for the folded q projection: rows 32*j..+16 of col-block j
    # hold scale*Wq[head] so lhsT/rhs partition bases match the k tiles.
    def wqt(heads, rows):
        out = np.zeros((rows, C * len(heads)), f32)
        for j, h in enumerate(heads):
            out[32 * j:32 * j + 16, C * j:C * (j + 1)] = Wq[16 * h:16 * h + 16, :]
        return cc(out)
    consts["wqTA"] = wqt((0, 1, 2), C)
    consts["wqTB"] = wqt((4, 5, 6), C)
    consts["wqTC"] = wqt((3, 7), 48)
    consts["wl"] = cc(np.asarray(lepe_lin_w, f32).T.copy())
    # kv linear output channel z*64 + 16*hh + d ; branch heads hh=0..3
    for br, Wk in ((1, Wk1), (2, Wk2)):
        consts[f"wk{br}a"] = padheads(Wk[0:64, :], (0, 1, 2))
        consts[f"wk{br}b"] = cc(Wk[48:64, :].T.copy())
        consts[f"wv{br}a"] = padheads(Wk[64:128, :], (0, 1, 2))
        consts[f"wv{br}b"] = cc(Wk[112:128, :].T.copy())

    # projection weights: lhsT rows 32*hh+d -> proj column of head (br,hh) dim d
    def projpad2(heads):
        out = np.zeros((C, C), f32)
        for hh, h in enumerate(heads):
            out[32 * hh:32 * hh + 16, :] = Pw[:, 16 * h:16 * h + 16].T
        return cc(out)
    consts["wpA"] = projpad2((0, 1, 2, 3))
    consts["wpB"] = projpad2((4, 5, 6, 7))
    consts["wpl"] = cc(Pw.T.copy())
    # recip-broadcast selector: pg rows 32*hh..+16 <- rc row 32*hh+16
    e8 = np.zeros((C, C), f32)
    for hh in range(4):
        e8[32 * hh + 16, 32 * hh:32 * hh + 16] = 1.0
    consts["e8n"] = e8
    cw = np.asarray(lepe_conv_w, f32)  # (C,1,3,3)
    for t in range(9):
        d9 = np.zeros((C, C), f32)
        d9[np.arange(C), np.arange(C)] = cw[:, 0, t // 3, t % 3]
        consts[f"diag{t}"] = d9
    consts["ident16"] = np.eye(C, dtype=f32)
    consts["wl16"] = consts.pop("wl")
    bi = np.zeros((C, 6 + C + 9), f32)
    bi[:, 0] = np.asarray(lepe_lin_b, f32).reshape(-1)
    bi[:, 1] = np.asarray(lepe_conv_b, f32).reshape(-1)
    bi[:, 2] = np.asarray(proj_b, f32).reshape(-1)
    bi[:, 3] = f32(np.asarray(f1_b).reshape(-1)[0])
    bi[:, 4] = f32(np.asarray(f2_b).reshape(-1)[0])
    bi[:, 5] = f32(np.asarray(f3_b).reshape(-1)[0])
    bi[:, 6:6 + C] = np.eye(C, dtype=f32)
    bi[:, 6 + C:6 + C + 9] = cw.reshape(C, 9)

    catg = np.zeros((C, CATG_COLS), bf16)
    for name, (o, w, rows) in CATG_OFF.items():
        catg[:rows, o:o + w] = consts[name]
    catj = np.zeros((C, CATJ_COLS), f32)
    for name, (o, w, rows) in CATJ_OFF.items():
        catj[:rows, o:o + w] = consts[name]
    cath = np.zeros((C, CATH_COLS), bf16)
    for name, (o, w, rows) in CATH_OFF.items():
        cath[:rows, o:o + w] = consts[name]
    return {"catg": cc(catg), "catj": cc(catj), "cath": cc(cath),
            "biasid": cc(bi)}


def _build_s1(idxb, f1_w, f2_w, f3_w):
    """Selection matrix turning seq1's sorted-gather + learned reduce into a
    plain matmul over raw x rows: seq1[tok] = sum_n S1[n, tok] * x[n]."""
    f32 = np.float32
    S = np.zeros((N, TT), f32)
    fw1 = np.asarray(f1_w, f32).reshape(-1)
    fw2 = np.asarray(f2_w, f32).reshape(-1)
    fw3 = np.asarray(f3_w, f32).reshape(-1)
    S[idxb[:N4], np.repeat(np.arange(TOK1), F1)] = np.tile(fw1, TOK1)
    S[idxb[N4:3 * N4], TOK1 + np.repeat(np.arange(TOK2), F2)] = np.tile(fw2, TOK2)
    S[idxb[3 * N4:], TOK1 + TOK2 + np.repeat(np.arange(TOK3), F3)] = np.tile(fw3, TOK3)
    return S.reshape(28, 112, TT).transpose(1, 0, 2).reshape(112, 28 * TT)


_RUN_KW = {}


def kernel(x, mask, H, W, W_q, W_kv1, W_kv2, f1_w, f1_b, f2_w, f2_b, f3_w, f3_b,
           lepe_lin_w, lepe_lin_b, lepe_conv_w, lepe_conv_b, proj_w, proj_b):
    x = np.ascontiguousarray(np.asarray(x, dtype=np.float32))
    mask = np.asarray(mask, dtype=np.float32)
    idx = np.argsort(mask.reshape(B, N), axis=1, kind="stable").astype(np.int32)

    consts = _host_consts(W_q, W_kv1, W_kv2, lepe_lin_w, lepe_lin_b, lepe_conv_w,
                          lepe_conv_b, proj_w, proj_b, f1_w, f1_b, f2_w, f2_b,
                          f3_w, f3_b)

    nc = _build_program()

    bf16 = ml_dtypes.bfloat16
    xb16 = x.astype(bf16)
    in_maps = []
    for core in range(NCORES):
        bs = core * BPC
        xloc = np.ascontiguousarray(xb16[bs:bs + BPC].reshape(BPC * N, C))
        iloc = (idx[bs:bs + BPC] + (np.arange(BPC)[:, None] * N).astype(np.int32))
        iloc = np.ascontiguousarray(iloc.reshape(BPC * N, 1))
        s1 = np.concatenate(
            [_build_s1(idx[bs + b], f1_w, f2_w, f3_w) for b in range(BPC)],
            axis=1).astype(bf16)
        m = {"xin": xloc, "idxin": iloc, "s1in": np.ascontiguousarray(s1)}
        m.update(consts)
        in_maps.append(m)

    res = run_bass_kernel_spmd(nc, in_maps, core_ids=list(range(NCORES)), **_RUN_KW)
    out = np.empty((B, N, C), np.float32)
    for core in range(NCORES):
        bs = core * BPC
        out[bs:bs + BPC] = res.results[core]["out"].reshape(BPC, N, C).astype(np.float32)
    kernel.last_result = res
    return out


# revision 68
# speedup vs baseline: 1.5496x; 1.0068x over previous
import os
import sys

if "/opt/trn_rl_repo" not in sys.path:
    sys.path.insert(0, "/opt/trn_rl_repo")

import ml_dtypes
import numpy as np

import concourse.bass as bass
import concourse.mybir as mybir
import concourse.tile as tile
from concourse import bacc
from concourse.bass_utils import run_bass_kernel_spmd

# Problem constants (hardcoded per harness contract).
B, Himg, Wimg, C, NH = 16, 56, 56, 128, 8
N = Himg * Wimg            # 3136
HD = C // NH               # 16
SCALE = HD ** -0.5         # 0.25
N4 = N // 4                # 784
TOK1 = (N // 49) // 4      # 16
TOK2 = (N // 14) // 2      # 112
TOK3 = (N // 7) // 4       # 112
TT = TOK1 + TOK2 + TOK3    # 240
F1, F2, F3 = 49, 14, 7
NCORES = 8
BPC = B // NCORES          # batches per core = 2
CH = 448                   # token chunk (448 = 8 rows of 56)
NCH = N // CH              # 7
RW = Wimg + 2              # padded row width 58
PADN = RW * (Himg + 2)     # 58*58 = 3364

FP32 = mybir.dt.float32
F32RT = mybir.dt.float32r
BF16 = mybir.dt.bfloat16
INT32 = mybir.dt.int32

# Head placement: all on-chip per-head blocks sit at 32-aligned partitions
# (HW requires 32-aligned partition starts; PE operands allow only 0/32/64).
# q/k tiles: A1 = heads 0,1,2 of branch1; A2 = heads 4,5,6 of branch2;
# B-tile "37": head 3 at rows 0:16, head 7 at rows 32:48.
# Attention epilogue: per-branch PSUM tile [128,448]; head hh of the branch
# occupies rows 32*hh..32*hh+15 (numerators) and row 32*hh+16 (softmax
# denominator via a ones column in the v tile); rows 17..31 of each 32-block
# are never written and hold 1.0 from a one-time memset.


STAGES = os.environ.get("STAGES", "ABCDEFGHIJ")


def _cat_layout(entries):
    off, table = 0, {}
    for name, w, rows in entries:
        table[name] = (off, w, rows)
        off += w
    return table, off


CATG_OFF, CATG_COLS = _cat_layout(
    [("wk1a", C, C), ("wk1b", 16, C), ("wv1a", C, C), ("wv1b", 16, C),
     ("wk2a", C, C), ("wk2b", 16, C), ("wv2a", C, C), ("wv2b", 16, C),
     ("wqTA", 3 * C, C), ("wqTB", 3 * C, C), ("wqTC", 2 * C, 48)])
CATJ_OFF, CATJ_COLS = _cat_layout(
    [("wpA", C, C), ("wpB", C, C), ("wpl", C, C), ("e8n", C, C)]
    + [(f"diag{t}", C, C) for t in range(9)])
CATH_OFF, CATH_COLS = _cat_layout([("ident16", C, C), ("wl16", C, C)])
S1W = 28 * TT


def _build_program():
    nc = bacc.Bacc(None, target_bir_lowering=False, debug=False)

    def din(name, shape, dt=FP32):
        return nc.dram_tensor(name, shape, dt, kind="ExternalInput")

    xin = din("xin", [BPC * N, C], BF16)
    idxin = din("idxin", [BPC * N, 1], INT32)
    catg = din("catg", [C, CATG_COLS], BF16)
    catj = din("catj", [C, CATJ_COLS], F32RT)
    cath = din("cath", [C, CATH_COLS], BF16)
    biasid = din("biasid", [C, 6 + C + 9], FP32)
    s1in = din("s1in", [112, BPC * S1W], BF16)
    outd = nc.dram_tensor("out", [BPC * N, C], BF16, kind="ExternalOutput")

    with tile.TileContext(nc) as tc:
        with (
            tc.tile_pool(name="const", bufs=1) as cpool,
            tc.tile_pool(name="big", bufs=1) as bigpool,
            tc.tile_pool(name="xtp", bufs=2) as xtpool,
            tc.tile_pool(name="xload", bufs=2) as xlpool,
            tc.tile_pool(name="xbig", bufs=2) as xbpool,
            tc.tile_pool(name="gath", bufs=1) as gpool,
            tc.tile_pool(name="small", bufs=2) as smpool,
            tc.tile_pool(name="expp", bufs=8) as epool,
            tc.tile_pool(name="opp", bufs=1) as oppool,
            tc.tile_pool(name="stream", bufs=2) as stpool,
            tc.tile_pool(name="ps_mm", bufs=2, space="PSUM") as ps_mm,
            tc.tile_pool(name="ps_pl", bufs=2, space="PSUM") as ps_pl,
            tc.tile_pool(name="ps_s1", bufs=1, space="PSUM") as ps_s1,
            tc.tile_pool(name="ps_po", bufs=2, space="PSUM") as ps_po,
            tc.tile_pool(name="ps_tp", bufs=1, space="PSUM") as ps_tp,
        ):
            # ---- constants: early (bf16 ident/wl + kv/q weights) now,
            # late (projection/conv weights) after the x/s1 loads ----
            ch_t = cpool.tile([C, CATH_COLS], BF16, tag="cath")
            cg_t = cpool.tile([C, CATG_COLS], BF16, tag="catg")
            cj_t = cpool.tile([C, CATJ_COLS], F32RT, tag="catj")
            bi_t = cpool.tile([C, 6 + C + 9], FP32, tag="c_b6")

            def emit_early_consts():
                nc.sync.dma_start(ch_t[:, :], cath[:, :])
                nc.sync.dma_start(cg_t[:, :], catg[:, :])
                nc.sync.dma_start(bi_t[:, :], biasid[:, :])

            def emit_late_consts():
                nc.sync.dma_start(cj_t[:, :], catj[:, :])

            def slC(tile_, table, name):
                o, w, rows = table[name]
                return tile_[:rows, o:o + w]

            id16_s = slC(ch_t, CATH_OFF, "ident16")
            wl_s = slC(ch_t, CATH_OFF, "wl16")
            wqTA_s, wqTB_s, wqTC_s = (slC(cg_t, CATG_OFF, n)
                                      for n in ("wqTA", "wqTB", "wqTC"))
            wk1a_s, wk1b_s = slC(cg_t, CATG_OFF, "wk1a"), slC(cg_t, CATG_OFF, "wk1b")
            wv1a_s, wv1b_s = slC(cg_t, CATG_OFF, "wv1a"), slC(cg_t, CATG_OFF, "wv1b")
            wk2a_s, wk2b_s = slC(cg_t, CATG_OFF, "wk2a"), slC(cg_t, CATG_OFF, "wk2b")
            wv2a_s, wv2b_s = slC(cg_t, CATG_OFF, "wv2a"), slC(cg_t, CATG_OFF, "wv2b")
            wpA_s, wpB_s, wpl_s = (slC(cj_t, CATJ_OFF, n)
                                   for n in ("wpA", "wpB", "wpl"))
            e8n_s = slC(cj_t, CATJ_OFF, "e8n")
            diag_s = [slC(cj_t, CATJ_OFF, f"diag{t}") for t in range(9)]
            b3_s = bi_t[:, 0:3]
            fb3_s = bi_t[:, 3:6]
            idf_s = bi_t[:, 6:6 + C]
            cw9_s = bi_t[:, 6 + C:6 + C + 9]

            F32R = mybir.dt.float32r

            def r(ap):
                return ap.bitcast(F32R)

            # persistent padded buffer for depthwise conv input (zero border)
            pad_t = bigpool.tile([C, PADN], F32RT, tag="pad")
            pad3 = pad_t[:, :].rearrange("p (r c) -> p r c", c=RW)

            # persistent per-branch attention accumulator tiles in SBUF;
            # rows 17..31 of each 32-block are never written and keep 1.0 so
            # the whole-tile reciprocal in stage J stays finite.
            op_att = [oppool.tile([C, CH], F32RT, tag=f"op{br}", name=f"opt{br}")
                      for br in range(2)]

            def emit_persistent_memsets():
                nc.gpsimd.memset(pad_t[:, :].bitcast(FP32), 0.0)
                for t_ in op_att:
                    nc.gpsimd.memset(t_[:, :].bitcast(FP32), 1.0)

            cp = nc.vector.tensor_copy

            def emit_pro_a(b, S):
                """Gather-dependent branch-2 chain + first x^T group."""
                xb = b * N
                S["xb"] = xb
                # ---- stage D: seq2 top-240 token gathers ----
                s2a = gpool.tile([128, C], BF16, tag="s2a")
                s2b = gpool.tile([112, C], BF16, tag="s2b")
                ita = xlpool.tile([128, 1], INT32, tag="ita")
                nc.sync.dma_start(ita[:, :], idxin[xb + N - TT: xb + N - TT + 128, :])
                nc.gpsimd.indirect_dma_start(
                    out=s2a[:, :], out_offset=None, in_=xin[:, :],
                    in_offset=bass.IndirectOffsetOnAxis(ap=ita[:, :1], axis=0))
                itb = xlpool.tile([112, 1], INT32, tag="itb")
                nc.sync.dma_start(itb[:, :], idxin[xb + N - 112: xb + N, :])
                nc.gpsimd.indirect_dma_start(
                    out=s2b[:, :], out_offset=None, in_=xin[:, :],
                    in_offset=bass.IndirectOffsetOnAxis(ap=itb[:, :1], axis=0))
                yield
                # ---- x / S1 loads ----
                xT = xtpool.tile([C, N], BF16, tag="xT")
                xsrc = xin[:, :].rearrange("(t p) c -> p t c", p=112)
                xt = xbpool.tile([112, 28 * C], BF16, tag="xt")
                s1_s = cpool.tile([112, S1W], BF16, tag="c_s1")
                for dq in range(4):
                    nc.sync.dma_start(
                        xt[:, 7 * C * dq:7 * C * (dq + 1)].rearrange(
                            "p (t c) -> p t c", c=C),
                        xsrc[:, 28 * b + 7 * dq:28 * b + 7 * (dq + 1), :])
                    lo = dq * 7 * TT
                    hi = min(S1W, (dq + 1) * 7 * TT)
                    nc.sync.dma_start(s1_s[:, lo:hi],
                                      s1in[:, b * S1W + lo:b * S1W + hi])
                    if dq == 0:
                        yield
                S.update(xT=xT, xt=xt, s1_s=s1_s, s2a=s2a, s2b=s2b)
                yield

                def emit_xtg(g):
                    tp = ps_tp.tile([C, 448], BF16, tag="tp")
                    for j in range(4):
                        t_ = 4 * g + j
                        nc.tensor.transpose(tp[:, 112 * j:112 * (j + 1)],
                                            xt[:, C * t_:C * (t_ + 1)],
                                            id16_s[:112, :112])
                    cp(xT[:, 448 * g:448 * (g + 1)], tp[:, :])
                S["emit_xtg"] = emit_xtg
                emit_xtg(0)
                # ---- stage F: seq2^T ----
                seq2T = smpool.tile([C, 256], BF16, tag="seq2T")
                nc.vector.memset(seq2T[:, 240:256], 0.0)
                pss = ps_tp.tile([C, TT], BF16, tag="tp")
                nc.tensor.transpose(pss[:, 0:128], s2a[:, :], id16_s[:, :])
                nc.tensor.transpose(pss[:, 128:TT], s2b[:, :], id16_s[:112, :112])
                cp(seq2T[:, 0:TT], pss[:, :])
                yield
                kp1a = smpool.tile([C, 256], BF16, tag="kp1a")
                kp2a = smpool.tile([C, 256], BF16, tag="kp2a")
                vp1a = smpool.tile([C, TT], FP32, tag="vp1a")
                vp2a = smpool.tile([C, TT], FP32, tag="vp2a")
                k37 = smpool.tile([48, 256], BF16, tag="k37")
                v37 = smpool.tile([48, TT], FP32, tag="v37")
                kqs = smpool.tile([C, NH * TT], BF16, tag="kqs")
                vaug = [[None, None], [None, None]]
                S.update(kp1a=kp1a, kp2a=kp2a, vp1a=vp1a, vp2a=vp2a, k37=k37,
                         v37=v37, kqs=kqs, vaug=vaug, seq1T=None, seq2T=seq2T)

                def emit_G(projs):
                    for (wt, seqT, dst, dp, m) in projs:
                        pk = ps_mm.tile([C, 256], FP32, tag="mm")
                        nc.tensor.matmul(out=pk[:m, :], lhsT=wt[:, :],
                                         rhs=seqT[:, 0:256], start=True, stop=True)
                        cp(dst[dp:dp + m, 0:TT], pk[:m, 0:TT])

                def emit_G2(heads):
                    # kq_h = (scale*Wq_h)^T k_h, so logits = kq_h^T xT
                    for h in heads:
                        hh = h % 4
                        if hh < 3:
                            kt = kp1a if h < 4 else kp2a
                            wt = wqTA_s if h < 4 else wqTB_s
                            kb, wc = 32 * hh, C * hh
                        else:
                            kt = k37
                            wt = wqTC_s
                            kb, wc = 32 * (h // 4), C * (h // 4)
                        pq = ps_mm.tile([C, 256], FP32, tag="mm", name="kq")
                        nc.tensor.matmul(out=pq[:, :],
                                         lhsT=wt[kb:kb + 16, wc:wc + C],
                                         rhs=kt[kb:kb + 16, 0:256],
                                         start=True, stop=True)
                        cp(kqs[:, TT * h:TT * (h + 1)], pq[:, 0:TT])

                def emit_H(br):
                    # vaug[br][mc] [mlen, 136]: head hh at cols 34*hh: 0:16 v,
                    # col 16 ones (-> denominator row 16 of the po matmul),
                    # cols 17:33 zero so a 33-wide lhsT keeps the PE tile legal.
                    vpa = vp1a if br == 0 else vp2a
                    for mc, (ms, ml) in enumerate(((0, 128), (128, 112))):
                        va = smpool.tile([128, 136], F32RT, tag=f"va{br}{mc}")
                        nc.gpsimd.memset(va[:ml, :].bitcast(FP32), 0.0)
                        nc.gpsimd.memset(va[:ml, 16::34].bitcast(FP32), 1.0)
                        vaug[br][mc] = va
                        for hh in range(4):
                            if hh < 3:
                                vsrc, sb = vpa, 32 * hh
                            else:
                                vsrc, sb = v37, 32 * br
                            pv = ps_po.tile([C, 16], FP32, tag="po", name="pv")
                            nc.tensor.transpose(pv[:ml, :],
                                                vsrc[sb:sb + 16, ms:ms + ml],
                                                idf_s[sb:sb + 16, sb:sb + 16])
                            cp(va[:ml, 34 * hh:34 * hh + 16], pv[:ml, :])
                S.update(emit_G=emit_G, emit_G2=emit_G2, emit_H=emit_H)
                # branch 2 (heads 4-7) only needs seq2T
                emit_G(((wk2a_s, seq2T, kp2a, 0, C), (wv2a_s, seq2T, vp2a, 0, C),
                        (wk2b_s, seq2T, k37, 32, 16), (wv2b_s, seq2T, v37, 32, 16)))
                yield
                emit_G2((4, 5))
                emit_G2((6, 7))
                yield
                emit_H(1)
                S["lepeT"] = bigpool.tile([C, N], F32RT, tag="lepeT", name="lepeT")

            def emit_pro_b(b, S):
                """S1-gated seq1 chain: remaining x^T groups, seq1, branch-1
                kv/kq/vaug. Drained as filler inside the chunk stream."""
                xt, s1_s = S["xt"], S["s1_s"]
                for g in range(1, 7):
                    S["emit_xtg"](g)
                    yield
                seq1T = smpool.tile([C, 256], BF16, tag="seq1T")
                S["seq1T"] = seq1T
                ps1 = ps_s1.tile([C, TT], FP32, tag="s1", name="ps1")
                for g in range(7):
                    for j in range(4):
                        t_ = 4 * g + j
                        nc.tensor.matmul(
                            out=ps1[:, :], lhsT=xt[:, C * t_:C * (t_ + 1)],
                            rhs=s1_s[:, TT * t_:TT * (t_ + 1)],
                            start=(t_ == 0), stop=(t_ == 27))
                    yield
                nc.vector.tensor_scalar_add(seq1T[:, 0:TOK1], ps1[:, 0:TOK1],
                                            fb3_s[:, 0:1])
                nc.vector.tensor_scalar_add(seq1T[:, TOK1:TOK1 + TOK2],
                                            ps1[:, TOK1:TOK1 + TOK2], fb3_s[:, 1:2])
                nc.vector.tensor_scalar_add(seq1T[:, TOK1 + TOK2:TT],
                                            ps1[:, TOK1 + TOK2:TT], fb3_s[:, 2:3])
                nc.vector.memset(seq1T[:, 240:256], 0.0)
                yield
                S["emit_G"](((wk1a_s, seq1T, S["kp1a"], 0, C),
                             (wv1a_s, seq1T, S["vp1a"], 0, C),
                             (wk1b_s, seq1T, S["k37"], 0, 16),
                             (wv1b_s, seq1T, S["v37"], 0, 16)))
                yield
                S["emit_G2"]((0, 1))
                yield
                S["emit_G2"]((2, 3))
                yield
                S["emit_H"](0)

            def chain2(g1, g2):
                yield from g1
                yield from g2

            def step(filler, n=1):
                if filler is None:
                    return
                for _ in range(n):
                    try:
                        next(filler)
                    except StopIteration:
                        return

            def emit_B(S, c):
                sl = slice(CH * c, CH * (c + 1))
                pl = ps_mm.tile([C, CH], FP32, tag="mm")
                nc.tensor.matmul(out=pl[:, :], lhsT=wl_s[:, :], rhs=S["xT"][:, sl],
                                 start=True, stop=True)
                pl3 = pl[:, :].rearrange("p (r c) -> p r c", c=Wimg)
                nc.vector.tensor_scalar_add(
                    pad3[:, 1 + 8 * c:9 + 8 * c, 1:57], pl3, b3_s[:, 0:1])

            def emit_C(S, c):
                pc = ps_mm.tile([C, CH], FP32, tag="mm")
                for t in range(9):
                    dy, dx = t // 3, t % 3
                    nc.tensor.matmul(
                        out=pc[:, :], lhsT=diag_s[t][:, :],
                        rhs=pad3[:, dy + 8 * c: dy + 8 * c + 8, dx: dx + Wimg],
                        start=(t == 0), stop=(t == 8))
                nc.vector.tensor_scalar_add(
                    S["lepeT"][:, CH * c:CH * (c + 1)], pc[:, :], b3_s[:, 1:2])

            def emit_logits(S, h, c):
                sl = slice(CH * c, CH * (c + 1))
                kqs = S["kqs"]
                pl0 = ps_pl.tile([C, CH], FP32, tag="pl", name="pl0")
                nc.tensor.matmul(out=pl0[:, :], lhsT=kqs[:, TT * h:TT * h + 128],
                                 rhs=S["xT"][:, sl], start=True, stop=True)
                pl1 = ps_pl.tile([112, CH], FP32, tag="pl", name="pl1")
                nc.tensor.matmul(out=pl1[:, :], lhsT=kqs[:, TT * h + 128:TT * (h + 1)],
                                 rhs=S["xT"][:, sl], start=True, stop=True)
                return pl0, pl1

            def emit_J_head(br):
                # Softmax division for a completed branch: reciprocal of
                # the assembled op tile -> row-broadcast matmul -> multiply.
                op = op_att[br]
                rc = stpool.tile([C, CH], F32RT, tag=f"rc{br}", name="rc")
                with nc.allow_low_precision(reason="f32r softmax recip"):
                    nc.vector.reciprocal(rc[:, :], op[:, :])
                pg = ps_mm.tile([C, CH], FP32, tag="mm", name="pg")
                nc.tensor.matmul(out=pg[:, :], lhsT=e8n_s[:, :], rhs=rc[:, :],
                                 start=True, stop=True)
                rpg = stpool.tile([C, CH], F32RT, tag=f"rp{br}")
                nc.vector.tensor_tensor(out=rpg[:, :], in0=op[:, :],
                                        in1=pg[:, :], op=mybir.AluOpType.mult)
                return rpg

            HORDER = (4, 5, 6, 7, 0, 1, 2, 3)

            def emit_chunk_I(S, c, pls, filler, boost=False):
                vaug = S["vaug"]
                prp = [None, None]
                for hi in range(NH):
                    h = HORDER[hi]
                    br = 0 if h < 4 else 1
                    hh = h % 4
                    pl0, pl1 = pls
                    if hi + 1 < NH:
                        pls = emit_logits(S, HORDER[hi + 1], c)
                    e0 = epool.tile([C, CH], F32RT, tag="e0", name="e0")
                    nc.scalar.activation(e0[:, :], pl0[:, :],
                                         mybir.ActivationFunctionType.Exp)
                    e1 = epool.tile([112, CH], F32RT, tag="e1", name="e1")
                    nc.scalar.activation(e1[:, :], pl1[:, :],
                                         mybir.ActivationFunctionType.Exp)
                    po = ps_po.tile([33, CH], FP32, tag="po", name="po")
                    nc.tensor.matmul(out=po[:, :],
                                     lhsT=vaug[br][0][:, 34 * hh:34 * hh + 33],
                                     rhs=e0[:, :], start=True, stop=False)
                    nc.tensor.matmul(out=po[:, :],
                                     lhsT=vaug[br][1][:112, 34 * hh:34 * hh + 33],
                                     rhs=e1[:, :], start=False, stop=True)
                    if h == 5:
                        nc.scalar.copy(
                            op_att[br][32 * hh:32 * hh + 17, :], po[0:17, :])
                    else:
                        nc.vector.tensor_copy(
                            op_att[br][32 * hh:32 * hh + 17, :], po[0:17, :])
                    if hh == 3:
                        prp[br] = emit_J_head(br)
                        if br == 0 and c + 1 < NCH:
                            pls = emit_logits(S, HORDER[0], c + 1)
                    if boost and hi < 3:
                        step(filler, 6)
                    elif h % 2 == 1:
                        step(filler)
                return prp, pls

            def emit_J_tail(S, c, prp):
                sl = slice(CH * c, CH * (c + 1))
                pp = ps_mm.tile([C, CH], FP32, tag="mm", name="pp")
                nc.tensor.matmul(out=pp[:, :], lhsT=wpA_s[:, :], rhs=r(prp[0][:, :]),
                                 start=True, stop=False)
                nc.tensor.matmul(out=pp[:, :], lhsT=wpB_s[:, :], rhs=r(prp[1][:, :]),
                                 start=False, stop=False)
                nc.tensor.matmul(out=pp[:, :], lhsT=wpl_s[:, :], rhs=S["lepeT"][:, sl],
                                 start=False, stop=True)
                sp = oppool.tile([C, CH], BF16, tag="sp", name="sp")
                nc.vector.tensor_scalar_add(sp[:, :], pp[:, :], b3_s[:, 2:3])
                so = xlpool.tile([112, 4 * 128], BF16, tag="so", name="so")
                pt2 = ps_tp.tile([112, 4 * 128], BF16, tag="tp", name="pt2")
                odst = outd[:, :].rearrange("(t p) c -> p t c", p=112)
                ot = (S["xb"] + CH * c) // 112
                halves = ((0, 2), (2, 4)) if c == NCH - 1 else ((0, 4),)
                for j0, j1 in halves:
                    for j in range(j0, j1):
                        nc.tensor.transpose(pt2[:, 128 * j:128 * (j + 1)],
                                            sp[:, 112 * j:112 * (j + 1)],
                                            id16_s[:, :])
                    nc.vector.tensor_copy(so[:, 128 * j0:128 * j1],
                                          pt2[:, 128 * j0:128 * j1])
                    nc.sync.dma_start(
                        odst[:, ot + j0:ot + j1, :],
                        so[:, 128 * j0:128 * j1].rearrange("p (t c) -> p t c", c=128))

            def emit_chunks(S, filler, boost=False):
                emit_B(S, 0)
                emit_B(S, 1)
                pls = emit_logits(S, HORDER[0], 0)
                for c in range(NCH):
                    if c == NCH - 1:
                        emit_C(S, c)
                    prp, pls = emit_chunk_I(S, c, pls, filler, boost and c == 0)
                    if c + 2 < NCH:
                        emit_B(S, c + 2)
                    if c < NCH - 1:
                        emit_C(S, c)
                    emit_J_tail(S, c, prp)
                    step(filler)

            S0, S1 = {}, {}
            gen0a = emit_pro_a(0, S0)
            step(gen0a, 1)
            emit_persistent_memsets()
            step(gen0a, 1)
            emit_early_consts()
            step(gen0a, 1)
            emit_late_consts()
            step(gen0a, 10 ** 6)
            step(emit_pro_b(0, S0), 10 ** 6)
            fill0 = chain2(emit_pro_a(1, S1), emit_pro_b(1, S1))
            emit_chunks(S0, fill0, boost=False)
            step(fill0, 10 ** 6)
            emit_chunks(S1, None)

    nc.compile()
    return nc


def _host_consts(W_q, W_kv1, W_kv2, lepe_lin_w, lepe_lin_b, lepe_conv_w, lepe_conv_b,
                 proj_w, proj_b, f1_w, f1_b, f2_w, f2_b, f3_w, f3_b):
    cc = np.ascontiguousarray
    f32 = np.float32
    bf16 = ml_dtypes.bfloat16
    consts = {}
    Wq = np.asarray(W_q, f32) * SCALE          # (C_out, C_in)
    Wk1 = np.asarray(W_kv1, f32)
    Wk2 = np.asarray(W_kv2, f32)
    Pw = np.asarray(proj_w, f32)

    def padheads(Wrows, heads):
        # lhsT [C_in, padded cols (16 used per 32-stride head)] producing padded rows
        out = np.zeros((C, C if len(heads) == 3 else 32 * len(heads)), f32)
        for lh, h in enumerate(heads):
            out[:, 32 * lh:32 * lh + 16] = Wrows[16 * h:16 * h + 16, :].T
        return cc(out)

    # wqT blocks for the folded q projection: rows 32*j..+16 of col-block j
    # hold scale*Wq[head] so lhsT/rhs partition bases match the k tiles.
    def wqt(heads, rows):
        out = np.zeros((rows, C * len(heads)), f32)
        for j, h in enumerate(heads):
            out[32 * j:32 * j + 16, C * j:C * (j + 1)] = Wq[16 * h:16 * h + 16, :]
        return cc(out)
    consts["wqTA"] = wqt((0, 1, 2), C)
    consts["wqTB"] = wqt((4, 5, 6), C)
    consts["wqTC"] = wqt((3, 7), 48)
    consts["wl"] = cc(np.asarray(lepe_lin_w, f32).T.copy())
    # kv linear output channel z*64 + 16*hh + d ; branch heads hh=0..3
    for br, Wk in ((1, Wk1), (2, Wk2)):
        consts[f"wk{br}a"] = padheads(Wk[0:64, :], (0, 1, 2))
        consts[f"wk{br}b"] = cc(Wk[48:64, :].T.copy())
        consts[f"wv{br}a"] = padheads(Wk[64:128, :], (0, 1, 2))
        consts[f"wv{br}b"] = cc(Wk[112:128, :].T.copy())

    # projection weights: lhsT rows 32*hh+d -> proj column of head (br,hh) dim d
    def projpad2(heads):
        out = np.zeros((C, C), f32)
        for hh, h in enumerate(heads):
            out[32 * hh:32 * hh + 16, :] = Pw[:, 16 * h:16 * h + 16].T
        return cc(out)
    consts["wpA"] = projpad2((0, 1, 2, 3))
    consts["wpB"] = projpad2((4, 5, 6, 7))
    consts["wpl"] = cc(Pw.T.copy())
    # recip-broadcast selector: pg rows 32*hh..+16 <- rc row 32*hh+16
    e8 = np.zeros((C, C), f32)
    for hh in range(4):
        e8[32 * hh + 16, 32 * hh:32 * hh + 16] = 1.0
    consts["e8n"] = e8
    cw = np.asarray(lepe_conv_w, f32)  # (C,1,3,3)
    for t in range(9):
        d9 = np.zeros((C, C), f32)
        d9[np.arange(C), np.arange(C)] = cw[:, 0, t // 3, t % 3]
        consts[f"diag{t}"] = d9
    consts["ident16"] = np.eye(C, dtype=f32)
    consts["wl16"] = consts.pop("wl")
    bi = np.zeros((C, 6 + C + 9), f32)
    bi[:, 0] = np.asarray(lepe_lin_b, f32).reshape(-1)
    bi[:, 1] = np.asarray(lepe_conv_b, f32).reshape(-1)
    bi[:, 2] = np.asarray(proj_b, f32).reshape(-1)
    bi[:, 3] = f32(np.asarray(f1_b).reshape(-1)[0])
    bi[:, 4] = f32(np.asarray(f2_b).reshape(-1)[0])
    bi[:, 5] = f32(np.asarray(f3_b).reshape(-1)[0])
    bi[:, 6:6 + C] = np.eye(C, dtype=f32)
    bi[:, 6 + C:6 + C + 9] = cw.reshape(C, 9)

    catg = np.zeros((C, CATG_COLS), bf16)
    for name, (o, w, rows) in CATG_OFF.items():
        catg[:rows, o:o + w] = consts[name]
    catj = np.zeros((C, CATJ_COLS), f32)
    for name, (o, w, rows) in CATJ_OFF.items():
        catj[:rows, o:o + w] = consts[name]
    cath = np.zeros((C, CATH_COLS), bf16)
    for name, (o, w, rows) in CATH_OFF.items():
        cath[:rows, o:o + w] = consts[name]
    return {"catg": cc(catg), "catj": cc(catj), "cath": cc(cath),
            "biasid": cc(bi)}


def _build_s1(idxb, f1_w, f2_w, f3_w):
    """Selection matrix turning seq1's sorted-gather + learned reduce into a
    plain matmul over raw x rows: seq1[tok] = sum_n S1[n, tok] * x[n]."""
    f32 = np.float32
    S = np.zeros((N, TT), f32)
    fw1 = np.asarray(f1_w, f32).reshape(-1)
    fw2 = np.asarray(f2_w, f32).reshape(-1)
    fw3 = np.asarray(f3_w, f32).reshape(-1)
    S[idxb[:N4], np.repeat(np.arange(TOK1), F1)] = np.tile(fw1, TOK1)
    S[idxb[N4:3 * N4], TOK1 + np.repeat(np.arange(TOK2), F2)] = np.tile(fw2, TOK2)
    S[idxb[3 * N4:], TOK1 + TOK2 + np.repeat(np.arange(TOK3), F3)] = np.tile(fw3, TOK3)
    return S.reshape(28, 112, TT).transpose(1, 0, 2).reshape(112, 28 * TT)


_RUN_KW = {}


def kernel(x, mask, H, W, W_q, W_kv1, W_kv2, f1_w, f1_b, f2_w, f2_b, f3_w, f3_b,
           lepe_lin_w, lepe_lin_b, lepe_conv_w, lepe_conv_b, proj_w, proj_b):
    x = np.ascontiguousarray(np.asarray(x, dtype=np.float32))
    mask = np.asarray(mask, dtype=np.float32)
    idx = np.argsort(mask.reshape(B, N), axis=1, kind="stable").astype(np.int32)

    consts = _host_consts(W_q, W_kv1, W_kv2, lepe_lin_w, lepe_lin_b, lepe_conv_w,
                          lepe_conv_b, proj_w, proj_b, f1_w, f1_b, f2_w, f2_b,
                          f3_w, f3_b)

    nc = _build_program()

    bf16 = ml_dtypes.bfloat16
    xb16 = x.astype(bf16)
    in_maps = []
    for core in range(NCORES):
        bs = core * BPC
        xloc = np.ascontiguousarray(xb16[bs:bs + BPC].reshape(BPC * N, C))
        iloc = (idx[bs:bs + BPC] + (np.arange(BPC)[:, None] * N).astype(np.int32))
        iloc = np.ascontiguousarray(iloc.reshape(BPC * N, 1))
        s1 = np.concatenate(
            [_build_s1(idx[bs + b], f1_w, f2_w, f3_w) for b in range(BPC)],
            axis=1).astype(bf16)
        m = {"xin": xloc, "idxin": iloc, "s1in": np.ascontiguousarray(s1)}
        m.update(consts)
        in_maps.append(m)

    res = run_bass_kernel_spmd(nc, in_maps, core_ids=list(range(NCORES)), **_RUN_KW)
    out = np.empty((B, N, C), np.float32)
    for core in range(NCORES):
        bs = core * BPC
        out[bs:bs + BPC] = res.results[core]["out"].reshape(BPC, N, C).astype(np.float32)
    kernel.last_result = res
    return out


# revision 69
# speedup vs baseline: 1.5538x; 1.0027x over previous
import os
import sys

if "/opt/trn_rl_repo" not in sys.path:
    sys.path.insert(0, "/opt/trn_rl_repo")

import ml_dtypes
import numpy as np

import concourse.bass as bass
import concourse.mybir as mybir
import concourse.tile as tile
from concourse import bacc
from concourse.bass_utils import run_bass_kernel_spmd

# Problem constants (hardcoded per harness contract).
B, Himg, Wimg, C, NH = 16, 56, 56, 128, 8
N = Himg * Wimg            # 3136
HD = C // NH               # 16
SCALE = HD ** -0.5         # 0.25
N4 = N // 4                # 784
TOK1 = (N // 49) // 4      # 16
TOK2 = (N // 14) // 2      # 112
TOK3 = (N // 7) // 4       # 112
TT = TOK1 + TOK2 + TOK3    # 240
F1, F2, F3 = 49, 14, 7
NCORES = 8
BPC = B // NCORES          # batches per core = 2
CH = 448                   # token chunk (448 = 8 rows of 56)
NCH = N // CH              # 7
RW = Wimg + 2              # padded row width 58
PADN = RW * (Himg + 2)     # 58*58 = 3364

FP32 = mybir.dt.float32
F32RT = mybir.dt.float32r
BF16 = mybir.dt.bfloat16
INT32 = mybir.dt.int32

# Head placement: all on-chip per-head blocks sit at 32-aligned partitions
# (HW requires 32-aligned partition starts; PE operands allow only 0/32/64).
# q/k tiles: A1 = heads 0,1,2 of branch1; A2 = heads 4,5,6 of branch2;
# B-tile "37": head 3 at rows 0:16, head 7 at rows 32:48.
# Attention epilogue: per-branch PSUM tile [128,448]; head hh of the branch
# occupies rows 32*hh..32*hh+15 (numerators) and row 32*hh+16 (softmax
# denominator via a ones column in the v tile); rows 17..31 of each 32-block
# are never written and hold 1.0 from a one-time memset.


STAGES = os.environ.get("STAGES", "ABCDEFGHIJ")


def _cat_layout(entries):
    off, table = 0, {}
    for name, w, rows in entries:
        table[name] = (off, w, rows)
        off += w
    return table, off


CATG_OFF, CATG_COLS = _cat_layout(
    [("wk1a", C, C), ("wk1b", 16, C), ("wv1a", C, C), ("wv1b", 16, C),
     ("wk2a", C, C), ("wk2b", 16, C), ("wv2a", C, C), ("wv2b", 16, C),
     ("wqTA", 3 * C, C), ("wqTB", 3 * C, C), ("wqTC", 2 * C, 48)])
CATJ_OFF, CATJ_COLS = _cat_layout(
    [("wpA", C, C), ("wpB", C, C), ("wpl", C, C), ("e8n", C, C)]
    + [(f"diag{t}", C, C) for t in range(9)])
CATH_OFF, CATH_COLS = _cat_layout([("ident16", C, C), ("wl16", C, C)])
S1W = 28 * TT


def _build_program():
    nc = bacc.Bacc(None, target_bir_lowering=False, debug=False)

    def din(name, shape, dt=FP32):
        return nc.dram_tensor(name, shape, dt, kind="ExternalInput")

    xin = din("xin", [BPC * N, C], BF16)
    idxin = din("idxin", [BPC * N, 1], INT32)
    catg = din("catg", [C, CATG_COLS], BF16)
    catj = din("catj", [C, CATJ_COLS], F32RT)
    cath = din("cath", [C, CATH_COLS], BF16)
    biasid = din("biasid", [C, 6 + C + 9], FP32)
    s1in = din("s1in", [112, BPC * S1W], BF16)
    outd = nc.dram_tensor("out", [BPC * N, C], BF16, kind="ExternalOutput")

    with tile.TileContext(nc) as tc:
        with (
            tc.tile_pool(name="const", bufs=1) as cpool,
            tc.tile_pool(name="big", bufs=1) as bigpool,
            tc.tile_pool(name="xtp", bufs=2) as xtpool,
            tc.tile_pool(name="xload", bufs=2) as xlpool,
            tc.tile_pool(name="xbig", bufs=2) as xbpool,
            tc.tile_pool(name="gath", bufs=1) as gpool,
            tc.tile_pool(name="small", bufs=2) as smpool,
            tc.tile_pool(name="expp", bufs=8) as epool,
            tc.tile_pool(name="opp", bufs=1) as oppool,
            tc.tile_pool(name="stream", bufs=2) as stpool,
            tc.tile_pool(name="ps_mm", bufs=2, space="PSUM") as ps_mm,
            tc.tile_pool(name="ps_pl", bufs=2, space="PSUM") as ps_pl,
            tc.tile_pool(name="ps_s1", bufs=1, space="PSUM") as ps_s1,
            tc.tile_pool(name="ps_po", bufs=2, space="PSUM") as ps_po,
            tc.tile_pool(name="ps_tp", bufs=1, space="PSUM") as ps_tp,
        ):
            # ---- constants: early (bf16 ident/wl + kv/q weights) now,
            # late (projection/conv weights) after the x/s1 loads ----
            ch_t = cpool.tile([C, CATH_COLS], BF16, tag="cath")
            cg_t = cpool.tile([C, CATG_COLS], BF16, tag="catg")
            cj_t = cpool.tile([C, CATJ_COLS], F32RT, tag="catj")
            bi_t = cpool.tile([C, 6 + C + 9], FP32, tag="c_b6")

            def emit_early_consts():
                nc.sync.dma_start(ch_t[:, :], cath[:, :])
                nc.sync.dma_start(cg_t[:, :], catg[:, :])
                nc.sync.dma_start(bi_t[:, :], biasid[:, :])

            def emit_late_consts():
                nc.sync.dma_start(cj_t[:, :], catj[:, :])

            def slC(tile_, table, name):
                o, w, rows = table[name]
                return tile_[:rows, o:o + w]

            id16_s = slC(ch_t, CATH_OFF, "ident16")
            wl_s = slC(ch_t, CATH_OFF, "wl16")
            wqTA_s, wqTB_s, wqTC_s = (slC(cg_t, CATG_OFF, n)
                                      for n in ("wqTA", "wqTB", "wqTC"))
            wk1a_s, wk1b_s = slC(cg_t, CATG_OFF, "wk1a"), slC(cg_t, CATG_OFF, "wk1b")
            wv1a_s, wv1b_s = slC(cg_t, CATG_OFF, "wv1a"), slC(cg_t, CATG_OFF, "wv1b")
            wk2a_s, wk2b_s = slC(cg_t, CATG_OFF, "wk2a"), slC(cg_t, CATG_OFF, "wk2b")
            wv2a_s, wv2b_s = slC(cg_t, CATG_OFF, "wv2a"), slC(cg_t, CATG_OFF, "wv2b")
            wpA_s, wpB_s, wpl_s = (slC(cj_t, CATJ_OFF, n)
                                   for n in ("wpA", "wpB", "wpl"))
            e8n_s = slC(cj_t, CATJ_OFF, "e8n")
            diag_s = [slC(cj_t, CATJ_OFF, f"diag{t}") for t in range(9)]
            b3_s = bi_t[:, 0:3]
            fb3_s = bi_t[:, 3:6]
            idf_s = bi_t[:, 6:6 + C]
            cw9_s = bi_t[:, 6 + C:6 + C + 9]

            F32R = mybir.dt.float32r

            def r(ap):
                return ap.bitcast(F32R)

            # persistent padded buffer for depthwise conv input (zero border)
            pad_t = bigpool.tile([C, PADN], F32RT, tag="pad")
            pad3 = pad_t[:, :].rearrange("p (r c) -> p r c", c=RW)

            # persistent per-branch attention accumulator tiles in SBUF;
            # rows 17..31 of each 32-block are never written and keep 1.0 so
            # the whole-tile reciprocal in stage J stays finite.
            op_att = [oppool.tile([C, CH], F32RT, tag=f"op{br}", name=f"opt{br}")
                      for br in range(2)]

            def emit_persistent_memsets():
                nc.gpsimd.memset(pad_t[:, :].bitcast(FP32), 0.0)
                for t_ in op_att:
                    nc.gpsimd.memset(t_[:, :].bitcast(FP32), 1.0)

            cp = nc.vector.tensor_copy

            def emit_pro_a(b, S):
                """Gather-dependent branch-2 chain + first x^T group."""
                xb = b * N
                S["xb"] = xb
                # ---- stage D: seq2 top-240 token gathers ----
                s2a = gpool.tile([128, C], BF16, tag="s2a")
                s2b = gpool.tile([112, C], BF16, tag="s2b")
                ita = xlpool.tile([128, 1], INT32, tag="ita")
                nc.sync.dma_start(ita[:, :], idxin[xb + N - TT: xb + N - TT + 128, :])
                nc.gpsimd.indirect_dma_start(
                    out=s2a[:, :], out_offset=None, in_=xin[:, :],
                    in_offset=bass.IndirectOffsetOnAxis(ap=ita[:, :1], axis=0))
                itb = xlpool.tile([112, 1], INT32, tag="itb")
                nc.sync.dma_start(itb[:, :], idxin[xb + N - 112: xb + N, :])
                nc.gpsimd.indirect_dma_start(
                    out=s2b[:, :], out_offset=None, in_=xin[:, :],
                    in_offset=bass.IndirectOffsetOnAxis(ap=itb[:, :1], axis=0))
                yield
                # ---- x / S1 loads ----
                xT = xtpool.tile([C, N], BF16, tag="xT")
                xsrc = xin[:, :].rearrange("(t p) c -> p t c", p=112)
                xt = xbpool.tile([112, 28 * C], BF16, tag="xt")
                s1_s = cpool.tile([112, S1W], BF16, tag="c_s1")
                for dq in range(4):
                    nc.sync.dma_start(
                        xt[:, 7 * C * dq:7 * C * (dq + 1)].rearrange(
                            "p (t c) -> p t c", c=C),
                        xsrc[:, 28 * b + 7 * dq:28 * b + 7 * (dq + 1), :])
                    lo = dq * 7 * TT
                    hi = min(S1W, (dq + 1) * 7 * TT)
                    nc.sync.dma_start(s1_s[:, lo:hi],
                                      s1in[:, b * S1W + lo:b * S1W + hi])
                    if dq == 0:
                        yield
                S.update(xT=xT, xt=xt, s1_s=s1_s, s2a=s2a, s2b=s2b)
                yield

                def emit_xtg(g):
                    tp = ps_tp.tile([C, 448], BF16, tag="tp")
                    for j in range(4):
                        t_ = 4 * g + j
                        nc.tensor.transpose(tp[:, 112 * j:112 * (j + 1)],
                                            xt[:, C * t_:C * (t_ + 1)],
                                            id16_s[:112, :112])
                    cp(xT[:, 448 * g:448 * (g + 1)], tp[:, :])
                S["emit_xtg"] = emit_xtg
                emit_xtg(0)
                # ---- stage F: seq2^T ----
                seq2T = smpool.tile([C, 256], BF16, tag="seq2T")
                nc.vector.memset(seq2T[:, 240:256], 0.0)
                pss = ps_tp.tile([C, TT], BF16, tag="tp")
                nc.tensor.transpose(pss[:, 0:128], s2a[:, :], id16_s[:, :])
                nc.tensor.transpose(pss[:, 128:TT], s2b[:, :], id16_s[:112, :112])
                cp(seq2T[:, 0:TT], pss[:, :])
                yield
                kp1a = smpool.tile([C, 256], BF16, tag="kp1a")
                kp2a = smpool.tile([C, 256], BF16, tag="kp2a")
                vp1a = smpool.tile([C, TT], FP32, tag="vp1a")
                vp2a = smpool.tile([C, TT], FP32, tag="vp2a")
                k37 = smpool.tile([48, 256], BF16, tag="k37")
                v37 = smpool.tile([48, TT], FP32, tag="v37")
                kqs = smpool.tile([C, NH * TT], BF16, tag="kqs")
                vaug = [[None, None], [None, None]]
                S.update(kp1a=kp1a, kp2a=kp2a, vp1a=vp1a, vp2a=vp2a, k37=k37,
                         v37=v37, kqs=kqs, vaug=vaug, seq1T=None, seq2T=seq2T)

                def emit_G(projs):
                    for (wt, seqT, dst, dp, m) in projs:
                        pk = ps_mm.tile([C, 256], FP32, tag="mm")
                        nc.tensor.matmul(out=pk[:m, :], lhsT=wt[:, :],
                                         rhs=seqT[:, 0:256], start=True, stop=True)
                        cp(dst[dp:dp + m, 0:TT], pk[:m, 0:TT])

                def emit_G2(heads):
                    # kq_h = (scale*Wq_h)^T k_h, so logits = kq_h^T xT
                    for h in heads:
                        hh = h % 4
                        if hh < 3:
                            kt = kp1a if h < 4 else kp2a
                            wt = wqTA_s if h < 4 else wqTB_s
                            kb, wc = 32 * hh, C * hh
                        else:
                            kt = k37
                            wt = wqTC_s
                            kb, wc = 32 * (h // 4), C * (h // 4)
                        pq = ps_mm.tile([C, 256], FP32, tag="mm", name="kq")
                        nc.tensor.matmul(out=pq[:, :],
                                         lhsT=wt[kb:kb + 16, wc:wc + C],
                                         rhs=kt[kb:kb + 16, 0:256],
                                         start=True, stop=True)
                        cp(kqs[:, TT * h:TT * (h + 1)], pq[:, 0:TT])

                def emit_H(br):
                    # vaug[br][mc] [mlen, 136]: head hh at cols 34*hh: 0:16 v,
                    # col 16 ones (-> denominator row 16 of the po matmul),
                    # cols 17:33 zero so a 33-wide lhsT keeps the PE tile legal.
                    vpa = vp1a if br == 0 else vp2a
                    for mc, (ms, ml) in enumerate(((0, 128), (128, 112))):
                        va = smpool.tile([128, 136], F32RT, tag=f"va{br}{mc}")
                        nc.gpsimd.memset(va[:ml, :].bitcast(FP32), 0.0)
                        nc.gpsimd.memset(va[:ml, 16::34].bitcast(FP32), 1.0)
                        vaug[br][mc] = va
                        for hh in range(4):
                            if hh < 3:
                                vsrc, sb = vpa, 32 * hh
                            else:
                                vsrc, sb = v37, 32 * br
                            pv = ps_po.tile([C, 16], FP32, tag="po", name="pv")
                            nc.tensor.transpose(pv[:ml, :],
                                                vsrc[sb:sb + 16, ms:ms + ml],
                                                idf_s[sb:sb + 16, sb:sb + 16])
                            cp(va[:ml, 34 * hh:34 * hh + 16], pv[:ml, :])
                S.update(emit_G=emit_G, emit_G2=emit_G2, emit_H=emit_H)
                # branch 2 (heads 4-7) only needs seq2T
                emit_G(((wk2a_s, seq2T, kp2a, 0, C), (wv2a_s, seq2T, vp2a, 0, C),
                        (wk2b_s, seq2T, k37, 32, 16), (wv2b_s, seq2T, v37, 32, 16)))
                yield
                emit_G2((4, 5))
                emit_G2((6, 7))
                yield
                emit_H(1)
                S["lepeT"] = bigpool.tile([C, N], F32RT, tag="lepeT", name="lepeT")

            def emit_pro_b(b, S):
                """S1-gated seq1 chain: remaining x^T groups, seq1, branch-1
                kv/kq/vaug. Drained as filler inside the chunk stream."""
                xt, s1_s = S["xt"], S["s1_s"]
                for g in range(1, 7):
                    S["emit_xtg"](g)
                    yield
                seq1T = smpool.tile([C, 256], BF16, tag="seq1T")
                S["seq1T"] = seq1T
                ps1 = ps_s1.tile([C, TT], FP32, tag="s1", name="ps1")
                for g in range(7):
                    for j in range(4):
                        t_ = 4 * g + j
                        nc.tensor.matmul(
                            out=ps1[:, :], lhsT=xt[:, C * t_:C * (t_ + 1)],
                            rhs=s1_s[:, TT * t_:TT * (t_ + 1)],
                            start=(t_ == 0), stop=(t_ == 27))
                    yield
                nc.vector.tensor_scalar_add(seq1T[:, 0:TOK1], ps1[:, 0:TOK1],
                                            fb3_s[:, 0:1])
                nc.vector.tensor_scalar_add(seq1T[:, TOK1:TOK1 + TOK2],
                                            ps1[:, TOK1:TOK1 + TOK2], fb3_s[:, 1:2])
                nc.vector.tensor_scalar_add(seq1T[:, TOK1 + TOK2:TT],
                                            ps1[:, TOK1 + TOK2:TT], fb3_s[:, 2:3])
                nc.vector.memset(seq1T[:, 240:256], 0.0)
                yield
                S["emit_G"](((wk1a_s, seq1T, S["kp1a"], 0, C),
                             (wv1a_s, seq1T, S["vp1a"], 0, C),
                             (wk1b_s, seq1T, S["k37"], 0, 16),
                             (wv1b_s, seq1T, S["v37"], 0, 16)))
                yield
                S["emit_G2"]((0, 1))
                yield
                S["emit_G2"]((2, 3))
                yield
                S["emit_H"](0)

            def chain2(g1, g2):
                yield from g1
                yield from g2

            def step(filler, n=1):
                if filler is None:
                    return
                for _ in range(n):
                    try:
                        next(filler)
                    except StopIteration:
                        return

            def emit_B(S, c):
                sl = slice(CH * c, CH * (c + 1))
                pl = ps_mm.tile([C, CH], FP32, tag="mm")
                nc.tensor.matmul(out=pl[:, :], lhsT=wl_s[:, :], rhs=S["xT"][:, sl],
                                 start=True, stop=True)
                pl3 = pl[:, :].rearrange("p (r c) -> p r c", c=Wimg)
                nc.vector.tensor_scalar_add(
                    pad3[:, 1 + 8 * c:9 + 8 * c, 1:57], pl3, b3_s[:, 0:1])

            def emit_C(S, c):
                pc = ps_mm.tile([C, CH], FP32, tag="mm")
                for t in range(9):
                    dy, dx = t // 3, t % 3
                    nc.tensor.matmul(
                        out=pc[:, :], lhsT=diag_s[t][:, :],
                        rhs=pad3[:, dy + 8 * c: dy + 8 * c + 8, dx: dx + Wimg],
                        start=(t == 0), stop=(t == 8))
                nc.vector.tensor_scalar_add(
                    S["lepeT"][:, CH * c:CH * (c + 1)], pc[:, :], b3_s[:, 1:2])

            def emit_logits(S, h, c):
                sl = slice(CH * c, CH * (c + 1))
                kqs = S["kqs"]
                pl0 = ps_pl.tile([C, CH], FP32, tag="pl", name="pl0")
                nc.tensor.matmul(out=pl0[:, :], lhsT=kqs[:, TT * h:TT * h + 128],
                                 rhs=S["xT"][:, sl], start=True, stop=True)
                pl1 = ps_pl.tile([112, CH], FP32, tag="pl", name="pl1")
                nc.tensor.matmul(out=pl1[:, :], lhsT=kqs[:, TT * h + 128:TT * (h + 1)],
                                 rhs=S["xT"][:, sl], start=True, stop=True)
                return pl0, pl1

            def emit_J_head(br):
                # Softmax division for a completed branch: reciprocal of
                # the assembled op tile -> row-broadcast matmul -> multiply.
                op = op_att[br]
                rc = stpool.tile([C, CH], F32RT, tag=f"rc{br}", name="rc")
                with nc.allow_low_precision(reason="f32r softmax recip"):
                    nc.vector.reciprocal(rc[:, :], op[:, :])
                pg = ps_mm.tile([C, CH], FP32, tag="mm", name="pg")
                nc.tensor.matmul(out=pg[:, :], lhsT=e8n_s[:, :], rhs=rc[:, :],
                                 start=True, stop=True)
                rpg = stpool.tile([C, CH], F32RT, tag=f"rp{br}")
                nc.vector.tensor_tensor(out=rpg[:, :], in0=op[:, :],
                                        in1=pg[:, :], op=mybir.AluOpType.mult)
                return rpg

            HORDER = (4, 5, 6, 7, 0, 1, 2, 3)

            def emit_chunk_I(S, c, pls, filler, boost=False):
                vaug = S["vaug"]
                prp = [None, None]
                for hi in range(NH):
                    h = HORDER[hi]
                    br = 0 if h < 4 else 1
                    hh = h % 4
                    pl0, pl1 = pls
                    if hi + 1 < NH:
                        pls = emit_logits(S, HORDER[hi + 1], c)
                    e0 = epool.tile([C, CH], F32RT, tag="e0", name="e0")
                    nc.scalar.activation(e0[:, :], pl0[:, :],
                                         mybir.ActivationFunctionType.Exp)
                    e1 = epool.tile([112, CH], F32RT, tag="e1", name="e1")
                    nc.scalar.activation(e1[:, :], pl1[:, :],
                                         mybir.ActivationFunctionType.Exp)
                    po = ps_po.tile([33, CH], FP32, tag="po", name="po")
                    nc.tensor.matmul(out=po[:, :],
                                     lhsT=vaug[br][0][:, 34 * hh:34 * hh + 33],
                                     rhs=e0[:, :], start=True, stop=False)
                    nc.tensor.matmul(out=po[:, :],
                                     lhsT=vaug[br][1][:112, 34 * hh:34 * hh + 33],
                                     rhs=e1[:, :], start=False, stop=True)
                    if h == 5:
                        nc.scalar.copy(
                            op_att[br][32 * hh:32 * hh + 17, :], po[0:17, :])
                    else:
                        nc.vector.tensor_copy(
                            op_att[br][32 * hh:32 * hh + 17, :], po[0:17, :])
                    if hh == 3:
                        prp[br] = emit_J_head(br)
                        if br == 0 and c + 1 < NCH:
                            pls = emit_logits(S, HORDER[0], c + 1)
                    if boost and hi < 3:
                        step(filler, 6)
                    elif h % 2 == 1:
                        step(filler)
                return prp, pls

            def emit_J_tail(S, c, prp):
                sl = slice(CH * c, CH * (c + 1))
                pp = ps_mm.tile([C, CH], FP32, tag="mm", name="pp")
                nc.tensor.matmul(out=pp[:, :], lhsT=wpB_s[:, :], rhs=r(prp[1][:, :]),
                                 start=True, stop=False)
                nc.tensor.matmul(out=pp[:, :], lhsT=wpl_s[:, :], rhs=S["lepeT"][:, sl],
                                 start=False, stop=False)
                nc.tensor.matmul(out=pp[:, :], lhsT=wpA_s[:, :], rhs=r(prp[0][:, :]),
                                 start=False, stop=True)
                sp = oppool.tile([C, CH], BF16, tag="sp", name="sp")
                nc.vector.tensor_scalar_add(sp[:, :], pp[:, :], b3_s[:, 2:3])
                so = xlpool.tile([112, 4 * 128], BF16, tag="so", name="so")
                pt2 = ps_tp.tile([112, 4 * 128], BF16, tag="tp", name="pt2")
                odst = outd[:, :].rearrange("(t p) c -> p t c", p=112)
                ot = (S["xb"] + CH * c) // 112
                halves = ((0, 2), (2, 4)) if c == NCH - 1 else ((0, 4),)
                for j0, j1 in halves:
                    for j in range(j0, j1):
                        nc.tensor.transpose(pt2[:, 128 * j:128 * (j + 1)],
                                            sp[:, 112 * j:112 * (j + 1)],
                                            id16_s[:, :])
                    nc.vector.tensor_copy(so[:, 128 * j0:128 * j1],
                                          pt2[:, 128 * j0:128 * j1])
                    nc.sync.dma_start(
                        odst[:, ot + j0:ot + j1, :],
                        so[:, 128 * j0:128 * j1].rearrange("p (t c) -> p t c", c=128))

            def emit_chunks(S, filler, boost=False):
                emit_B(S, 0)
                emit_B(S, 1)
                pls = emit_logits(S, HORDER[0], 0)
                for c in range(NCH):
                    if c == NCH - 1:
                        emit_C(S, c)
                    prp, pls = emit_chunk_I(S, c, pls, filler, boost and c == 0)
                    if c + 2 < NCH:
                        emit_B(S, c + 2)
                    if c < NCH - 1:
                        emit_C(S, c)
                    emit_J_tail(S, c, prp)
                    step(filler)

            S0, S1 = {}, {}
            gen0a = emit_pro_a(0, S0)
            step(gen0a, 1)
            emit_persistent_memsets()
            step(gen0a, 1)
            emit_early_consts()
            step(gen0a, 1)
            emit_late_consts()
            step(gen0a, 10 ** 6)
            step(emit_pro_b(0, S0), 10 ** 6)
            fill0 = chain2(emit_pro_a(1, S1), emit_pro_b(1, S1))
            emit_chunks(S0, fill0, boost=False)
            step(fill0, 10 ** 6)
            emit_chunks(S1, None)

    nc.compile()
    return nc


def _host_consts(W_q, W_kv1, W_kv2, lepe_lin_w, lepe_lin_b, lepe_conv_w, lepe_conv_b,
                 proj_w, proj_b, f1_w, f1_b, f2_w, f2_b, f3_w, f3_b):
    cc = np.ascontiguousarray
    f32 = np.float32
    bf16 = ml_dtypes.bfloat16
    consts = {}
    Wq = np.asarray(W_q, f32) * SCALE          # (C_out, C_in)
    Wk1 = np.asarray(W_kv1, f32)
    Wk2 = np.asarray(W_kv2, f32)
    Pw = np.asarray(proj_w, f32)

    def padheads(Wrows, heads):
        # lhsT [C_in, padded cols (16 used per 32-stride head)] producing padded rows
        out = np.zeros((C, C if len(heads) == 3 else 32 * len(heads)), f32)
        for lh, h in enumerate(heads):
            out[:, 32 * lh:32 * lh + 16] = Wrows[16 * h:16 * h + 16, :].T
        return cc(out)

    # wqT blocks for the folded q projection: rows 32*j..+16 of col-block j
    # hold scale*Wq[head] so lhsT/rhs partition bases match the k tiles.
    def wqt(heads, rows):
        out = np.zeros((rows, C * len(heads)), f32)
        for j, h in enumerate(heads):
            out[32 * j:32 * j + 16, C * j:C * (j + 1)] = Wq[16 * h:16 * h + 16, :]
        return cc(out)
    consts["wqTA"] = wqt((0, 1, 2), C)
    consts["wqTB"] = wqt((4, 5, 6), C)
    consts["wqTC"] = wqt((3, 7), 48)
    consts["wl"] = cc(np.asarray(lepe_lin_w, f32).T.copy())
    # kv linear output channel z*64 + 16*hh + d ; branch heads hh=0..3
    for br, Wk in ((1, Wk1), (2, Wk2)):
        consts[f"wk{br}a"] = padheads(Wk[0:64, :], (0, 1, 2))
        consts[f"wk{br}b"] = cc(Wk[48:64, :].T.copy())
        consts[f"wv{br}a"] = padheads(Wk[64:128, :], (0, 1, 2))
        consts[f"wv{br}b"] = cc(Wk[112:128, :].T.copy())

    # projection weights: lhsT rows 32*hh+d -> proj column of head (br,hh) dim d
    def projpad2(heads):
        out = np.zeros((C, C), f32)
        for hh, h in enumerate(heads):
            out[32 * hh:32 * hh + 16, :] = Pw[:, 16 * h:16 * h + 16].T
        return cc(out)
    consts["wpA"] = projpad2((0, 1, 2, 3))
    consts["wpB"] = projpad2((4, 5, 6, 7))
    consts["wpl"] = cc(Pw.T.copy())
    # recip-broadcast selector: pg rows 32*hh..+16 <- rc row 32*hh+16
    e8 = np.zeros((C, C), f32)
    for hh in range(4):
        e8[32 * hh + 16, 32 * hh:32 * hh + 16] = 1.0
    consts["e8n"] = e8
    cw = np.asarray(lepe_conv_w, f32)  # (C,1,3,3)
    for t in range(9):
        d9 = np.zeros((C, C), f32)
        d9[np.arange(C), np.arange(C)] = cw[:, 0, t // 3, t % 3]
        consts[f"diag{t}"] = d9
    consts["ident16"] = np.eye(C, dtype=f32)
    consts["wl16"] = consts.pop("wl")
    bi = np.zeros((C, 6 + C + 9), f32)
    bi[:, 0] = np.asarray(lepe_lin_b, f32).reshape(-1)
    bi[:, 1] = np.asarray(lepe_conv_b, f32).reshape(-1)
    bi[:, 2] = np.asarray(proj_b, f32).reshape(-1)
    bi[:, 3] = f32(np.asarray(f1_b).reshape(-1)[0])
    bi[:, 4] = f32(np.asarray(f2_b).reshape(-1)[0])
    bi[:, 5] = f32(np.asarray(f3_b).reshape(-1)[0])
    bi[:, 6:6 + C] = np.eye(C, dtype=f32)
    bi[:, 6 + C:6 + C + 9] = cw.reshape(C, 9)

    catg = np.zeros((C, CATG_COLS), bf16)
    for name, (o, w, rows) in CATG_OFF.items():
        catg[:rows, o:o + w] = consts[name]
    catj = np.zeros((C, CATJ_COLS), f32)
    for name, (o, w, rows) in CATJ_OFF.items():
        catj[:rows, o:o + w] = consts[name]
    cath = np.zeros((C, CATH_COLS), bf16)
    for name, (o, w, rows) in CATH_OFF.items():
        cath[:rows, o:o + w] = consts[name]
    return {"catg": cc(catg), "catj": cc(catj), "cath": cc(cath),
            "biasid": cc(bi)}


def _build_s1(idxb, f1_w, f2_w, f3_w):
    """Selection matrix turning seq1's sorted-gather + learned reduce into a
    plain matmul over raw x rows: seq1[tok] = sum_n S1[n, tok] * x[n]."""
    f32 = np.float32
    S = np.zeros((N, TT), f32)
    fw1 = np.asarray(f1_w, f32).reshape(-1)
    fw2 = np.asarray(f2_w, f32).reshape(-1)
    fw3 = np.asarray(f3_w, f32).reshape(-1)
    S[idxb[:N4], np.repeat(np.arange(TOK1), F1)] = np.tile(fw1, TOK1)
    S[idxb[N4:3 * N4], TOK1 + np.repeat(np.arange(TOK2), F2)] = np.tile(fw2, TOK2)
    S[idxb[3 * N4:], TOK1 + TOK2 + np.repeat(np.arange(TOK3), F3)] = np.tile(fw3, TOK3)
    return S.reshape(28, 112, TT).transpose(1, 0, 2).reshape(112, 28 * TT)


_RUN_KW = {}


def kernel(x, mask, H, W, W_q, W_kv1, W_kv2, f1_w, f1_b, f2_w, f2_b, f3_w, f3_b,
           lepe_lin_w, lepe_lin_b, lepe_conv_w, lepe_conv_b, proj_w, proj_b):
    x = np.ascontiguousarray(np.asarray(x, dtype=np.float32))
    mask = np.asarray(mask, dtype=np.float32)
    idx = np.argsort(mask.reshape(B, N), axis=1, kind="stable").astype(np.int32)

    consts = _host_consts(W_q, W_kv1, W_kv2, lepe_lin_w, lepe_lin_b, lepe_conv_w,
                          lepe_conv_b, proj_w, proj_b, f1_w, f1_b, f2_w, f2_b,
                          f3_w, f3_b)

    nc = _build_program()

    bf16 = ml_dtypes.bfloat16
    xb16 = x.astype(bf16)
    in_maps = []
    for core in range(NCORES):
        bs = core * BPC
        xloc = np.ascontiguousarray(xb16[bs:bs + BPC].reshape(BPC * N, C))
        iloc = (idx[bs:bs + BPC] + (np.arange(BPC)[:, None] * N).astype(np.int32))
        iloc = np.ascontiguousarray(iloc.reshape(BPC * N, 1))
        s1 = np.concatenate(
            [_build_s1(idx[bs + b], f1_w, f2_w, f3_w) for b in range(BPC)],
            axis=1).astype(bf16)
        m = {"xin": xloc, "idxin": iloc, "s1in": np.ascontiguousarray(s1)}
        m.update(consts)
        in_maps.append(m)

    res = run_bass_kernel_spmd(nc, in_maps, core_ids=list(range(NCORES)), **_RUN_KW)
    out = np.empty((B, N, C), np.float32)
    for core in range(NCORES):
        bs = core * BPC
        out[bs:bs + BPC] = res.results[core]["out"].reshape(BPC, N, C).astype(np.float32)
    kernel.last_result = res
    return out


# revision 70
# speedup vs baseline: 1.5928x; 1.0251x over previous
import os
import sys

if "/opt/trn_rl_repo" not in sys.path:
    sys.path.insert(0, "/opt/trn_rl_repo")

import ml_dtypes
import numpy as np

import concourse.bass as bass
import concourse.mybir as mybir
import concourse.tile as tile
from concourse import bacc
from concourse.bass_utils import run_bass_kernel_spmd

# Problem constants (hardcoded per harness contract).
B, Himg, Wimg, C, NH = 16, 56, 56, 128, 8
N = Himg * Wimg            # 3136
HD = C // NH               # 16
SCALE = HD ** -0.5         # 0.25
N4 = N // 4                # 784
TOK1 = (N // 49) // 4      # 16
TOK2 = (N // 14) // 2      # 112
TOK3 = (N // 7) // 4       # 112
TT = TOK1 + TOK2 + TOK3    # 240
F1, F2, F3 = 49, 14, 7
NCORES = 8
BPC = B // NCORES          # batches per core = 2
CH = 448                   # token chunk (448 = 8 rows of 56)
NCH = N // CH              # 7
RW = Wimg + 2              # padded row width 58
PADN = RW * (Himg + 2)     # 58*58 = 3364

FP32 = mybir.dt.float32
F32RT = mybir.dt.float32r
BF16 = mybir.dt.bfloat16
INT32 = mybir.dt.int32

# Head placement: all on-chip per-head blocks sit at 32-aligned partitions
# (HW requires 32-aligned partition starts; PE operands allow only 0/32/64).
# q/k tiles: A1 = heads 0,1,2 of branch1; A2 = heads 4,5,6 of branch2;
# B-tile "37": head 3 at rows 0:16, head 7 at rows 32:48.
# Attention epilogue: per-branch PSUM tile [128,448]; head hh of the branch
# occupies rows 32*hh..32*hh+15 (numerators) and row 32*hh+16 (softmax
# denominator via a ones column in the v tile); rows 17..31 of each 32-block
# are never written and hold 1.0 from a one-time memset.


STAGES = os.environ.get("STAGES", "ABCDEFGHIJ")


def _cat_layout(entries):
    off, table = 0, {}
    for name, w, rows in entries:
        table[name] = (off, w, rows)
        off += w
    return table, off


CATG_OFF, CATG_COLS = _cat_layout(
    [("wk1a", C, C), ("wk1b", 16, C), ("wv1a", C, C), ("wv1b", 16, C),
     ("wk2a", C, C), ("wk2b", 16, C), ("wv2a", C, C), ("wv2b", 16, C),
     ("wqTA", 3 * C, C), ("wqTB", 3 * C, C), ("wqTC", 2 * C, 48)])
CATJ_OFF, CATJ_COLS = _cat_layout(
    [("wpA", C, C), ("wpB", C, C), ("wpl", C, C), ("e8n", C, C)]
    + [(f"diag{t}", C, C) for t in range(9)])
CATH_OFF, CATH_COLS = _cat_layout([("ident16", C, C), ("wl16", C, C)])
S1W = 28 * TT


def _build_program():
    nc = bacc.Bacc(None, target_bir_lowering=False, debug=False)

    def din(name, shape, dt=FP32):
        return nc.dram_tensor(name, shape, dt, kind="ExternalInput")

    xin = din("xin", [BPC * N, C], BF16)
    idxin = din("idxin", [BPC * N, 1], INT32)
    catg = din("catg", [C, CATG_COLS], BF16)
    catj = din("catj", [C, CATJ_COLS], F32RT)
    cath = din("cath", [C, CATH_COLS], BF16)
    biasid = din("biasid", [C, 6 + C + 9], FP32)
    s1in = din("s1in", [112, BPC * S1W], BF16)
    outd = nc.dram_tensor("out", [BPC * N, C], BF16, kind="ExternalOutput")

    with tile.TileContext(nc) as tc:
        with (
            tc.tile_pool(name="const", bufs=1) as cpool,
            tc.tile_pool(name="big", bufs=1) as bigpool,
            tc.tile_pool(name="xtp", bufs=2) as xtpool,
            tc.tile_pool(name="xload", bufs=2) as xlpool,
            tc.tile_pool(name="xbig", bufs=2) as xbpool,
            tc.tile_pool(name="gath", bufs=1) as gpool,
            tc.tile_pool(name="small", bufs=2) as smpool,
            tc.tile_pool(name="expp", bufs=8) as epool,
            tc.tile_pool(name="opp", bufs=1) as oppool,
            tc.tile_pool(name="stream", bufs=2) as stpool,
            tc.tile_pool(name="ps_mm", bufs=2, space="PSUM") as ps_mm,
            tc.tile_pool(name="ps_pl", bufs=3, space="PSUM") as ps_pl,
            tc.tile_pool(name="ps_po", bufs=2, space="PSUM") as ps_po,
            tc.tile_pool(name="ps_tp", bufs=1, space="PSUM") as ps_tp,
        ):
            # ---- constants: early (bf16 ident/wl + kv/q weights) now,
            # late (projection/conv weights) after the x/s1 loads ----
            ch_t = cpool.tile([C, CATH_COLS], BF16, tag="cath")
            cg_t = cpool.tile([C, CATG_COLS], BF16, tag="catg")
            cj_t = cpool.tile([C, CATJ_COLS], F32RT, tag="catj")
            bi_t = cpool.tile([C, 6 + C + 9], FP32, tag="c_b6")

            def emit_early_consts():
                nc.sync.dma_start(ch_t[:, :], cath[:, :])
                nc.sync.dma_start(cg_t[:, :], catg[:, :])
                nc.sync.dma_start(bi_t[:, :], biasid[:, :])

            def emit_late_consts():
                nc.sync.dma_start(cj_t[:, :], catj[:, :])

            def slC(tile_, table, name):
                o, w, rows = table[name]
                return tile_[:rows, o:o + w]

            id16_s = slC(ch_t, CATH_OFF, "ident16")
            wl_s = slC(ch_t, CATH_OFF, "wl16")
            wqTA_s, wqTB_s, wqTC_s = (slC(cg_t, CATG_OFF, n)
                                      for n in ("wqTA", "wqTB", "wqTC"))
            wk1a_s, wk1b_s = slC(cg_t, CATG_OFF, "wk1a"), slC(cg_t, CATG_OFF, "wk1b")
            wv1a_s, wv1b_s = slC(cg_t, CATG_OFF, "wv1a"), slC(cg_t, CATG_OFF, "wv1b")
            wk2a_s, wk2b_s = slC(cg_t, CATG_OFF, "wk2a"), slC(cg_t, CATG_OFF, "wk2b")
            wv2a_s, wv2b_s = slC(cg_t, CATG_OFF, "wv2a"), slC(cg_t, CATG_OFF, "wv2b")
            wpA_s, wpB_s, wpl_s = (slC(cj_t, CATJ_OFF, n)
                                   for n in ("wpA", "wpB", "wpl"))
            e8n_s = slC(cj_t, CATJ_OFF, "e8n")
            diag_s = [slC(cj_t, CATJ_OFF, f"diag{t}") for t in range(9)]
            b3_s = bi_t[:, 0:3]
            fb3_s = bi_t[:, 3:6]
            idf_s = bi_t[:, 6:6 + C]
            cw9_s = bi_t[:, 6 + C:6 + C + 9]

            F32R = mybir.dt.float32r

            def r(ap):
                return ap.bitcast(F32R)

            # persistent padded buffer for depthwise conv input (zero border)
            pad_t = bigpool.tile([C, PADN], F32RT, tag="pad")
            pad3 = pad_t[:, :].rearrange("p (r c) -> p r c", c=RW)

            # persistent per-branch attention accumulator tiles in SBUF;
            # rows 17..31 of each 32-block are never written and keep 1.0 so
            # the whole-tile reciprocal in stage J stays finite.
            op_att = [oppool.tile([C, CH], F32RT, tag=f"op{br}", name=f"opt{br}")
                      for br in range(2)]

            def emit_persistent_memsets():
                nc.gpsimd.memset(pad_t[:, :].bitcast(FP32), 0.0)
                for t_ in op_att:
                    nc.gpsimd.memset(t_[:, :].bitcast(FP32), 1.0)

            cp = nc.vector.tensor_copy

            def emit_pro_a(b, S):
                """Gather-dependent branch-2 chain + first x^T group."""
                xb = b * N
                S["xb"] = xb
                # ---- stage D: seq2 top-240 token gathers ----
                s2a = gpool.tile([128, C], BF16, tag="s2a")
                s2b = gpool.tile([112, C], BF16, tag="s2b")
                ita = xlpool.tile([128, 1], INT32, tag="ita")
                nc.sync.dma_start(ita[:, :], idxin[xb + N - TT: xb + N - TT + 128, :])
                nc.gpsimd.indirect_dma_start(
                    out=s2a[:, :], out_offset=None, in_=xin[:, :],
                    in_offset=bass.IndirectOffsetOnAxis(ap=ita[:, :1], axis=0))
                itb = xlpool.tile([112, 1], INT32, tag="itb")
                nc.sync.dma_start(itb[:, :], idxin[xb + N - 112: xb + N, :])
                nc.gpsimd.indirect_dma_start(
                    out=s2b[:, :], out_offset=None, in_=xin[:, :],
                    in_offset=bass.IndirectOffsetOnAxis(ap=itb[:, :1], axis=0))
                yield
                # ---- x / S1 loads ----
                xT = xtpool.tile([C, N], BF16, tag="xT")
                xsrc = xin[:, :].rearrange("(t p) c -> p t c", p=112)
                xt = xbpool.tile([112, 28 * C], BF16, tag="xt")
                s1_s = cpool.tile([112, S1W], BF16, tag="c_s1")
                for dq in range(4):
                    nc.sync.dma_start(
                        xt[:, 7 * C * dq:7 * C * (dq + 1)].rearrange(
                            "p (t c) -> p t c", c=C),
                        xsrc[:, 28 * b + 7 * dq:28 * b + 7 * (dq + 1), :])
                    lo = dq * 7 * TT
                    hi = min(S1W, (dq + 1) * 7 * TT)
                    nc.sync.dma_start(s1_s[:, lo:hi],
                                      s1in[:, b * S1W + lo:b * S1W + hi])
                    if dq == 0:
                        yield
                S.update(xT=xT, xt=xt, s1_s=s1_s, s2a=s2a, s2b=s2b)
                yield

                def emit_xtg(g):
                    tp = ps_tp.tile([C, 448], BF16, tag="tp")
                    for j in range(4):
                        t_ = 4 * g + j
                        nc.tensor.transpose(tp[:, 112 * j:112 * (j + 1)],
                                            xt[:, C * t_:C * (t_ + 1)],
                                            id16_s[:112, :112])
                    cp(xT[:, 448 * g:448 * (g + 1)], tp[:, :])
                S["emit_xtg"] = emit_xtg
                emit_xtg(0)
                # ---- stage F: seq2^T ----
                seq2T = smpool.tile([C, 256], BF16, tag="seq2T")
                nc.vector.memset(seq2T[:, 240:256], 0.0)
                pss = ps_tp.tile([C, TT], BF16, tag="tp")
                nc.tensor.transpose(pss[:, 0:128], s2a[:, :], id16_s[:, :])
                nc.tensor.transpose(pss[:, 128:TT], s2b[:, :], id16_s[:112, :112])
                cp(seq2T[:, 0:TT], pss[:, :])
                yield
                kp1a = smpool.tile([C, 256], BF16, tag="kp1a")
                kp2a = smpool.tile([C, 256], BF16, tag="kp2a")
                vp1a = smpool.tile([C, TT], FP32, tag="vp1a")
                vp2a = smpool.tile([C, TT], FP32, tag="vp2a")
                k37 = smpool.tile([48, 256], BF16, tag="k37")
                v37 = smpool.tile([48, TT], FP32, tag="v37")
                kqs = smpool.tile([C, NH * TT], BF16, tag="kqs")
                vaug = [[None, None], [None, None]]
                S.update(kp1a=kp1a, kp2a=kp2a, vp1a=vp1a, vp2a=vp2a, k37=k37,
                         v37=v37, kqs=kqs, vaug=vaug, seq1T=None, seq2T=seq2T)

                def emit_G(projs):
                    for (wt, seqT, dst, dp, m) in projs:
                        pk = ps_mm.tile([C, 256], FP32, tag="mm")
                        nc.tensor.matmul(out=pk[:m, :], lhsT=wt[:, :],
                                         rhs=seqT[:, 0:256], start=True, stop=True)
                        cp(dst[dp:dp + m, 0:TT], pk[:m, 0:TT])

                def emit_G2(heads):
                    # kq_h = (scale*Wq_h)^T k_h, so logits = kq_h^T xT
                    for h in heads:
                        hh = h % 4
                        if hh < 3:
                            kt = kp1a if h < 4 else kp2a
                            wt = wqTA_s if h < 4 else wqTB_s
                            kb, wc = 32 * hh, C * hh
                        else:
                            kt = k37
                            wt = wqTC_s
                            kb, wc = 32 * (h // 4), C * (h // 4)
                        pq = ps_mm.tile([C, 256], FP32, tag="mm", name="kq")
                        nc.tensor.matmul(out=pq[:, :],
                                         lhsT=wt[kb:kb + 16, wc:wc + C],
                                         rhs=kt[kb:kb + 16, 0:256],
                                         start=True, stop=True)
                        cp(kqs[:, TT * h:TT * (h + 1)], pq[:, 0:TT])

                def emit_H(br):
                    # vaug[br][mc] [mlen, 136]: head hh at cols 34*hh: 0:16 v,
                    # col 16 ones (-> denominator row 16 of the po matmul),
                    # cols 17:33 zero so a 33-wide lhsT keeps the PE tile legal.
                    vpa = vp1a if br == 0 else vp2a
                    for mc, (ms, ml) in enumerate(((0, 128), (128, 112))):
                        va = smpool.tile([128, 136], F32RT, tag=f"va{br}{mc}")
                        nc.gpsimd.memset(va[:ml, :].bitcast(FP32), 0.0)
                        nc.gpsimd.memset(va[:ml, 16::34].bitcast(FP32), 1.0)
                        vaug[br][mc] = va
                        for hh in range(4):
                            if hh < 3:
                                vsrc, sb = vpa, 32 * hh
                            else:
                                vsrc, sb = v37, 32 * br
                            pv = ps_po.tile([C, 16], FP32, tag="po", name="pv")
                            nc.tensor.transpose(pv[:ml, :],
                                                vsrc[sb:sb + 16, ms:ms + ml],
                                                idf_s[sb:sb + 16, sb:sb + 16])
                            cp(va[:ml, 34 * hh:34 * hh + 16], pv[:ml, :])
                S.update(emit_G=emit_G, emit_G2=emit_G2, emit_H=emit_H)
                # branch 2 (heads 4-7) only needs seq2T
                emit_G(((wk2a_s, seq2T, kp2a, 0, C), (wv2a_s, seq2T, vp2a, 0, C),
                        (wk2b_s, seq2T, k37, 32, 16), (wv2b_s, seq2T, v37, 32, 16)))
                yield
                emit_G2((4, 5))
                emit_G2((6, 7))
                yield
                emit_H(1)
                S["lepeT"] = bigpool.tile([C, N], F32RT, tag="lepeT", name="lepeT")

            def emit_pro_b(b, S):
                """S1-gated seq1 chain: remaining x^T groups, seq1, branch-1
                kv/kq/vaug. Drained as filler inside the chunk stream."""
                xt, s1_s = S["xt"], S["s1_s"]
                for g in range(1, 7):
                    S["emit_xtg"](g)
                    yield
                seq1T = smpool.tile([C, 256], BF16, tag="seq1T")
                S["seq1T"] = seq1T
                ps1 = ps_po.tile([C, TT], FP32, tag="po", name="ps1")
                for t_ in range(28):
                    nc.tensor.matmul(
                        out=ps1[:, :], lhsT=xt[:, C * t_:C * (t_ + 1)],
                        rhs=s1_s[:, TT * t_:TT * (t_ + 1)],
                        start=(t_ == 0), stop=(t_ == 27))
                nc.vector.tensor_scalar_add(seq1T[:, 0:TOK1], ps1[:, 0:TOK1],
                                            fb3_s[:, 0:1])
                nc.vector.tensor_scalar_add(seq1T[:, TOK1:TOK1 + TOK2],
                                            ps1[:, TOK1:TOK1 + TOK2], fb3_s[:, 1:2])
                nc.vector.tensor_scalar_add(seq1T[:, TOK1 + TOK2:TT],
                                            ps1[:, TOK1 + TOK2:TT], fb3_s[:, 2:3])
                nc.vector.memset(seq1T[:, 240:256], 0.0)
                yield
                S["emit_G"](((wk1a_s, seq1T, S["kp1a"], 0, C),
                             (wv1a_s, seq1T, S["vp1a"], 0, C),
                             (wk1b_s, seq1T, S["k37"], 0, 16),
                             (wv1b_s, seq1T, S["v37"], 0, 16)))
                yield
                S["emit_G2"]((0, 1))
                yield
                S["emit_G2"]((2, 3))
                yield
                S["emit_H"](0)

            def chain2(g1, g2):
                yield from g1
                yield from g2

            def step(filler, n=1):
                if filler is None:
                    return
                for _ in range(n):
                    try:
                        next(filler)
                    except StopIteration:
                        return

            def emit_B(S, c):
                sl = slice(CH * c, CH * (c + 1))
                pl = ps_mm.tile([C, CH], FP32, tag="mm")
                nc.tensor.matmul(out=pl[:, :], lhsT=wl_s[:, :], rhs=S["xT"][:, sl],
                                 start=True, stop=True)
                pl3 = pl[:, :].rearrange("p (r c) -> p r c", c=Wimg)
                nc.vector.tensor_scalar_add(
                    pad3[:, 1 + 8 * c:9 + 8 * c, 1:57], pl3, b3_s[:, 0:1])

            def emit_C(S, c):
                pc = ps_mm.tile([C, CH], FP32, tag="mm")
                for t in range(9):
                    dy, dx = t // 3, t % 3
                    nc.tensor.matmul(
                        out=pc[:, :], lhsT=diag_s[t][:, :],
                        rhs=pad3[:, dy + 8 * c: dy + 8 * c + 8, dx: dx + Wimg],
                        start=(t == 0), stop=(t == 8))
                nc.vector.tensor_scalar_add(
                    S["lepeT"][:, CH * c:CH * (c + 1)], pc[:, :], b3_s[:, 1:2])

            def emit_logits(S, h, c):
                sl = slice(CH * c, CH * (c + 1))
                kqs = S["kqs"]
                pl0 = ps_pl.tile([C, CH], FP32, tag="pl", name="pl0")
                nc.tensor.matmul(out=pl0[:, :], lhsT=kqs[:, TT * h:TT * h + 128],
                                 rhs=S["xT"][:, sl], start=True, stop=True)
                pl1 = ps_pl.tile([112, CH], FP32, tag="pl", name="pl1")
                nc.tensor.matmul(out=pl1[:, :], lhsT=kqs[:, TT * h + 128:TT * (h + 1)],
                                 rhs=S["xT"][:, sl], start=True, stop=True)
                return pl0, pl1

            def emit_J_head(br):
                # Softmax division for a completed branch: reciprocal of
                # the assembled op tile -> row-broadcast matmul -> multiply.
                op = op_att[br]
                rc = stpool.tile([C, CH], F32RT, tag=f"rc{br}", name="rc")
                with nc.allow_low_precision(reason="f32r softmax recip"):
                    nc.vector.reciprocal(rc[:, :], op[:, :])
                pg = ps_mm.tile([C, CH], FP32, tag="mm", name="pg")
                nc.tensor.matmul(out=pg[:, :], lhsT=e8n_s[:, :], rhs=rc[:, :],
                                 start=True, stop=True)
                rpg = stpool.tile([C, CH], F32RT, tag=f"rp{br}")
                nc.vector.tensor_tensor(out=rpg[:, :], in0=op[:, :],
                                        in1=pg[:, :], op=mybir.AluOpType.mult)
                return rpg

            HORDER = (4, 5, 6, 7, 0, 1, 2, 3)

            def emit_chunk_I(S, c, pls, filler, boost=False):
                vaug = S["vaug"]
                prp = [None, None]
                for hi in range(NH):
                    h = HORDER[hi]
                    br = 0 if h < 4 else 1
                    hh = h % 4
                    pl0, pl1 = pls
                    if hi + 1 < NH:
                        pls = emit_logits(S, HORDER[hi + 1], c)
                    e0 = epool.tile([C, CH], F32RT, tag="e0", name="e0")
                    nc.scalar.activation(e0[:, :], pl0[:, :],
                                         mybir.ActivationFunctionType.Exp)
                    e1 = epool.tile([112, CH], F32RT, tag="e1", name="e1")
                    nc.scalar.activation(e1[:, :], pl1[:, :],
                                         mybir.ActivationFunctionType.Exp)
                    po = ps_po.tile([33, CH], FP32, tag="po", name="po")
                    nc.tensor.matmul(out=po[:, :],
                                     lhsT=vaug[br][0][:, 34 * hh:34 * hh + 33],
                                     rhs=e0[:, :], start=True, stop=False)
                    nc.tensor.matmul(out=po[:, :],
                                     lhsT=vaug[br][1][:112, 34 * hh:34 * hh + 33],
                                     rhs=e1[:, :], start=False, stop=True)
                    if h == 5:
                        nc.scalar.copy(
                            op_att[br][32 * hh:32 * hh + 17, :], po[0:17, :])
                    else:
                        nc.vector.tensor_copy(
                            op_att[br][32 * hh:32 * hh + 17, :], po[0:17, :])
                    if hh == 3:
                        prp[br] = emit_J_head(br)
                        if br == 0 and c + 1 < NCH:
                            pls = emit_logits(S, HORDER[0], c + 1)
                    if boost and hi < 3:
                        step(filler, 6)
                    elif h % 2 == 1:
                        step(filler)
                return prp, pls

            def emit_J_tail(S, c, prp):
                sl = slice(CH * c, CH * (c + 1))
                pp = ps_mm.tile([C, CH], FP32, tag="mm", name="pp")
                nc.tensor.matmul(out=pp[:, :], lhsT=wpB_s[:, :], rhs=r(prp[1][:, :]),
                                 start=True, stop=False)
                nc.tensor.matmul(out=pp[:, :], lhsT=wpl_s[:, :], rhs=S["lepeT"][:, sl],
                                 start=False, stop=False)
                nc.tensor.matmul(out=pp[:, :], lhsT=wpA_s[:, :], rhs=r(prp[0][:, :]),
                                 start=False, stop=True)
                sp = oppool.tile([C, CH], BF16, tag="sp", name="sp")
                nc.vector.tensor_scalar_add(sp[:, :], pp[:, :], b3_s[:, 2:3])
                so = xlpool.tile([112, 4 * 128], BF16, tag="so", name="so")
                pt2 = ps_tp.tile([112, 4 * 128], BF16, tag="tp", name="pt2")
                odst = outd[:, :].rearrange("(t p) c -> p t c", p=112)
                ot = (S["xb"] + CH * c) // 112
                halves = ((0, 2), (2, 4)) if c == NCH - 1 else ((0, 4),)
                for j0, j1 in halves:
                    for j in range(j0, j1):
                        nc.tensor.transpose(pt2[:, 128 * j:128 * (j + 1)],
                                            sp[:, 112 * j:112 * (j + 1)],
                                            id16_s[:, :])
                    nc.vector.tensor_copy(so[:, 128 * j0:128 * j1],
                                          pt2[:, 128 * j0:128 * j1])
                    nc.sync.dma_start(
                        odst[:, ot + j0:ot + j1, :],
                        so[:, 128 * j0:128 * j1].rearrange("p (t c) -> p t c", c=128))

            def emit_chunks(S, filler, boost=False):
                emit_B(S, 0)
                emit_B(S, 1)
                pls = emit_logits(S, HORDER[0], 0)
                for c in range(NCH):
                    if c == NCH - 1:
                        emit_C(S, c)
                    prp, pls = emit_chunk_I(S, c, pls, filler, boost and c == 0)
                    if c + 2 < NCH:
                        emit_B(S, c + 2)
                    if c < NCH - 1:
                        emit_C(S, c)
                    emit_J_tail(S, c, prp)
                    step(filler)

            S0, S1 = {}, {}
            gen0a = emit_pro_a(0, S0)
            step(gen0a, 1)
            emit_persistent_memsets()
            step(gen0a, 1)
            emit_early_consts()
            step(gen0a, 1)
            emit_late_consts()
            step(gen0a, 10 ** 6)
            step(emit_pro_b(0, S0), 10 ** 6)
            fill0 = chain2(emit_pro_a(1, S1), emit_pro_b(1, S1))
            emit_chunks(S0, fill0, boost=False)
            step(fill0, 10 ** 6)
            emit_chunks(S1, None)

    nc.compile()
    return nc


def _host_consts(W_q, W_kv1, W_kv2, lepe_lin_w, lepe_lin_b, lepe_conv_w, lepe_conv_b,
                 proj_w, proj_b, f1_w, f1_b, f2_w, f2_b, f3_w, f3_b):
    cc = np.ascontiguousarray
    f32 = np.float32
    bf16 = ml_dtypes.bfloat16
    consts = {}
    Wq = np.asarray(W_q, f32) * SCALE          # (C_out, C_in)
    Wk1 = np.asarray(W_kv1, f32)
    Wk2 = np.asarray(W_kv2, f32)
    Pw = np.asarray(proj_w, f32)

    def padheads(Wrows, heads):
        # lhsT [C_in, padded cols (16 used per 32-stride head)] producing padded rows
        out = np.zeros((C, C if len(heads) == 3 else 32 * len(heads)), f32)
        for lh, h in enumerate(heads):
            out[:, 32 * lh:32 * lh + 16] = Wrows[16 * h:16 * h + 16, :].T
        return cc(out)

    # wqT blocks for the folded q projection: rows 32*j..+16 of col-block j
    # hold scale*Wq[head] so lhsT/rhs partition bases match the k tiles.
    def wqt(heads, rows):
        out = np.zeros((rows, C * len(heads)), f32)
        for j, h in enumerate(heads):
            out[32 * j:32 * j + 16, C * j:C * (j + 1)] = Wq[16 * h:16 * h + 16, :]
        return cc(out)
    consts["wqTA"] = wqt((0, 1, 2), C)
    consts["wqTB"] = wqt((4, 5, 6), C)
    consts["wqTC"] = wqt((3, 7), 48)
    consts["wl"] = cc(np.asarray(lepe_lin_w, f32).T.copy())
    # kv linear output channel z*64 + 16*hh + d ; branch heads hh=0..3
    for br, Wk in ((1, Wk1), (2, Wk2)):
        consts[f"wk{br}a"] = padheads(Wk[0:64, :], (0, 1, 2))
        consts[f"wk{br}b"] = cc(Wk[48:64, :].T.copy())
        consts[f"wv{br}a"] = padheads(Wk[64:128, :], (0, 1, 2))
        consts[f"wv{br}b"] = cc(Wk[112:128, :].T.copy())

    # projection weights: lhsT rows 32*hh+d -> proj column of head (br,hh) dim d
    def projpad2(heads):
        out = np.zeros((C, C), f32)
        for hh, h in enumerate(heads):
            out[32 * hh:32 * hh + 16, :] = Pw[:, 16 * h:16 * h + 16].T
        return cc(out)
    consts["wpA"] = projpad2((0, 1, 2, 3))
    consts["wpB"] = projpad2((4, 5, 6, 7))
    consts["wpl"] = cc(Pw.T.copy())
    # recip-broadcast selector: pg rows 32*hh..+16 <- rc row 32*hh+16
    e8 = np.zeros((C, C), f32)
    for hh in range(4):
        e8[32 * hh + 16, 32 * hh:32 * hh + 16] = 1.0
    consts["e8n"] = e8
    cw = np.asarray(lepe_conv_w, f32)  # (C,1,3,3)
    for t in range(9):
        d9 = np.zeros((C, C), f32)
        d9[np.arange(C), np.arange(C)] = cw[:, 0, t // 3, t % 3]
        consts[f"diag{t}"] = d9
    consts["ident16"] = np.eye(C, dtype=f32)
    consts["wl16"] = consts.pop("wl")
    bi = np.zeros((C, 6 + C + 9), f32)
    bi[:, 0] = np.asarray(lepe_lin_b, f32).reshape(-1)
    bi[:, 1] = np.asarray(lepe_conv_b, f32).reshape(-1)
    bi[:, 2] = np.asarray(proj_b, f32).reshape(-1)
    bi[:, 3] = f32(np.asarray(f1_b).reshape(-1)[0])
    bi[:, 4] = f32(np.asarray(f2_b).reshape(-1)[0])
    bi[:, 5] = f32(np.asarray(f3_b).reshape(-1)[0])
    bi[:, 6:6 + C] = np.eye(C, dtype=f32)
    bi[:, 6 + C:6 + C + 9] = cw.reshape(C, 9)

    catg = np.zeros((C, CATG_COLS), bf16)
    for name, (o, w, rows) in CATG_OFF.items():
        catg[:rows, o:o + w] = consts[name]
    catj = np.zeros((C, CATJ_COLS), f32)
    for name, (o, w, rows) in CATJ_OFF.items():
        catj[:rows, o:o + w] = consts[name]
    cath = np.zeros((C, CATH_COLS), bf16)
    for name, (o, w, rows) in CATH_OFF.items():
        cath[:rows, o:o + w] = consts[name]
    return {"catg": cc(catg), "catj": cc(catj), "cath": cc(cath),
            "biasid": cc(bi)}


def _build_s1(idxb, f1_w, f2_w, f3_w):
    """Selection matrix turning seq1's sorted-gather + learned reduce into a
    plain matmul over raw x rows: seq1[tok] = sum_n S1[n, tok] * x[n]."""
    f32 = np.float32
    S = np.zeros((N, TT), f32)
    fw1 = np.asarray(f1_w, f32).reshape(-1)
    fw2 = np.asarray(f2_w, f32).reshape(-1)
    fw3 = np.asarray(f3_w, f32).reshape(-1)
    S[idxb[:N4], np.repeat(np.arange(TOK1), F1)] = np.tile(fw1, TOK1)
    S[idxb[N4:3 * N4], TOK1 + np.repeat(np.arange(TOK2), F2)] = np.tile(fw2, TOK2)
    S[idxb[3 * N4:], TOK1 + TOK2 + np.repeat(np.arange(TOK3), F3)] = np.tile(fw3, TOK3)
    return S.reshape(28, 112, TT).transpose(1, 0, 2).reshape(112, 28 * TT)


_RUN_KW = {}


def kernel(x, mask, H, W, W_q, W_kv1, W_kv2, f1_w, f1_b, f2_w, f2_b, f3_w, f3_b,
           lepe_lin_w, lepe_lin_b, lepe_conv_w, lepe_conv_b, proj_w, proj_b):
    x = np.ascontiguousarray(np.asarray(x, dtype=np.float32))
    mask = np.asarray(mask, dtype=np.float32)
    idx = np.argsort(mask.reshape(B, N), axis=1, kind="stable").astype(np.int32)

    consts = _host_consts(W_q, W_kv1, W_kv2, lepe_lin_w, lepe_lin_b, lepe_conv_w,
                          lepe_conv_b, proj_w, proj_b, f1_w, f1_b, f2_w, f2_b,
                          f3_w, f3_b)

    nc = _build_program()

    bf16 = ml_dtypes.bfloat16
    xb16 = x.astype(bf16)
    in_maps = []
    for core in range(NCORES):
        bs = core * BPC
        xloc = np.ascontiguousarray(xb16[bs:bs + BPC].reshape(BPC * N, C))
        iloc = (idx[bs:bs + BPC] + (np.arange(BPC)[:, None] * N).astype(np.int32))
        iloc = np.ascontiguousarray(iloc.reshape(BPC * N, 1))
        s1 = np.concatenate(
            [_build_s1(idx[bs + b], f1_w, f2_w, f3_w) for b in range(BPC)],
            axis=1).astype(bf16)
        m = {"xin": xloc, "idxin": iloc, "s1in": np.ascontiguousarray(s1)}
        m.update(consts)
        in_maps.append(m)

    res = run_bass_kernel_spmd(nc, in_maps, core_ids=list(range(NCORES)), **_RUN_KW)
    out = np.empty((B, N, C), np.float32)
    for core in range(NCORES):
        bs = core * BPC
        out[bs:bs + BPC] = res.results[core]["out"].reshape(BPC, N, C).astype(np.float32)
    kernel.last_result = res
    return out
